# revision 24
# baseline (speedup 1.0000x reference)
"""CrossScaleAttention Trainium2 kernel (v3).

Data-parallel over batch: 16 samples / 8 cores = 2 samples per core.

Algebraic restructuring (exact up to fp rounding):
  - 1x1 convs commute with nearest 2x upsample -> main branch runs at 32x32.
  - W_align folded into main qkv; pos/rel/biases folded into constants.
  - attention score via AUGMENTED EIGENDECOMPOSITION:
      score = x^T A x + u.x = [x;1]^T Btil [x;1],  Btil symmetric (C+1)^2.
      eigh(Btil), keep C largest-|lam| comps (drops ~1e-6 residual):
      score = sum_i sgn_i * (W_i.x + c_i)^2 + const(dropped by softmax).
    Square runs on the Activation engine with c as per-partition bias.
  - attn scales pixels (columns), so it commutes through the proj GEMM:
      Wf @ ((Wv x + cv) * attn) = (Wf Wv) @ (x * attn) + (Wf cv) (x) attn
    One fused GEMM (Wc = Wf Wv) on x' = x*attn (a 2x-rate bf16 DVE multiply)
    plus a rank-1 matmul (lhsT = Wf cv, rhs = attn row) into the same PSUM.
  - bout folded into the small-branch combine stt; main fmb = (amb*wfcm)+pf.
  - small-branch pixels host-permuted to (dh, h2l, dw, wj) so the softmax
    transpose DMAs are flat copies and the 2x2 upsample-add segments are
    contiguous-innermost views of PSUM / fmb.
  - bf16 inputs/weights/attn/outputs; f32r square/score path (t' needs f32
    mantissa); matmuls accumulate in f32 PSUM.
  - out-DMAs emitted after all input DMAs on the in-order SP queue so their
    waits never block input prefetch.
"""
import sys
sys.path.insert(0, '/opt/trn_rl_repo')
import numpy as np
import ml_dtypes

B, CS, CM = 16, 256, 512
HS = WS = 64
HM = WM = 32
NPX_S = HS * WS          # 4096
NPX_M = HM * WM          # 1024
NCORES = 8
BPC = B // NCORES        # 2 samples per core
CHUNK = 1024             # small-branch pixel chunk (16 h-rows)
NCHUNK = NPX_S // CHUNK  # 4

BF16 = ml_dtypes.bfloat16


# pixel permutation within a chunk: px' = (dh, h2l, dw, wj), px = h*64+w with
# h = c*16 + 2*h2l + dh, w = 2*wj + dw  (c = chunk index)
def _chunk_perm():
    dh, h2l, dw, wj = np.meshgrid(np.arange(2), np.arange(8), np.arange(2),
                                  np.arange(32), indexing='ij')
    h = 2 * h2l + dh
    w = 2 * wj + dw
    return (h * 64 + w).reshape(-1)


_PERM_CHUNK = _chunk_perm()
PERM = np.concatenate([c * CHUNK + _PERM_CHUNK for c in range(NCHUNK)])  # [4096]

_prog = None


def _build_program():
    import concourse.bacc as bacc
    import concourse.mybir as mybir
    from concourse.tile import TileContext

    f32, f32r, bf16 = mybir.dt.float32, mybir.dt.float32r, mybir.dt.bfloat16
    Act = mybir.ActivationFunctionType
    Alu = mybir.AluOpType

    nc = bacc.Bacc(None, target_bir_lowering=False)

    xs_d = nc.dram_tensor("xs", [BPC, 128, 2, NPX_S], bf16, kind="ExternalInput")
    xm_d = nc.dram_tensor("xm", [BPC, 128, 4 * NPX_M], bf16, kind="ExternalInput")
    WmT_d = nc.dram_tensor("WmT", [128, 4 * 512], bf16, kind="ExternalInput")
    WcmT_d = nc.dram_tensor("WcmT", [128, 4 * 256], bf16, kind="ExternalInput")
    WsT_d = nc.dram_tensor("WsT", [128, 2 * 256], bf16, kind="ExternalInput")
    WcsT_d = nc.dram_tensor("WcsT", [128, 2 * 256], bf16, kind="ExternalInput")
    sgm_d = nc.dram_tensor("sgm", [128, 4], f32r, kind="ExternalInput")
    cm_d = nc.dram_tensor("cm", [128, 4], f32, kind="ExternalInput")
    wfcm_d = nc.dram_tensor("wfcm", [128, 2], f32, kind="ExternalInput")
    bout_d = nc.dram_tensor("bout", [128, 2], f32, kind="ExternalInput")
    sgs_d = nc.dram_tensor("sgs", [128, 2], f32r, kind="ExternalInput")
    cs_d = nc.dram_tensor("cs", [128, 2], f32, kind="ExternalInput")
    wfcs_d = nc.dram_tensor("wfcs", [1, 256], bf16, kind="ExternalInput")
    out_d = nc.dram_tensor("out", [BPC, 128, 2, NPX_S], bf16, kind="ExternalOutput")

    with TileContext(nc) as tc:
        with (
            tc.tile_pool(name="wp", bufs=1) as wp,
            tc.tile_pool(name="mp", bufs=1) as mp,
            tc.tile_pool(name="sp", bufs=1) as sp,
            tc.tile_pool(name="ps_y", bufs=3, space="PSUM") as ps_y,
            tc.tile_pool(name="ps_f", bufs=3, space="PSUM") as ps_f,
            tc.tile_pool(name="ps_s", bufs=2, space="PSUM") as ps_s,
        ):
            # ---- resident weights; k-interleaved so the first matmul can
            # start after ~380KB instead of the full main working set
            WmT = wp.tile([128, 4 * 512], bf16, tag="WmT")
            xmt = []
            for b in range(BPC):
                t = mp.tile([128, 4 * NPX_M], bf16, tag="xm", bufs=2, name=f"xm{b}")
                xmt.append(t)
            for k in range(4):
                nc.sync.dma_start(out=WmT[:, k * 512:(k + 1) * 512],
                                  in_=WmT_d[:, k * 512:(k + 1) * 512])
                nc.sync.dma_start(out=xmt[0][:, k * NPX_M:(k + 1) * NPX_M],
                                  in_=xm_d[0, :, k * NPX_M:(k + 1) * NPX_M])

            def vec(dram, cols, name, dt):
                t = wp.tile([128, cols], dt, tag=name)
                nc.sync.dma_start(out=t[:], in_=dram[:])
                return t

            sgm = vec(sgm_d, 4, "sgm", f32r)
            cm = vec(cm_d, 4, "cm", f32)
            # small-branch score weights early: lets the scheduler fill main-
            # phase PE bubbles with chunk-0 GEMMs
            WsT = wp.tile([128, 2 * 256], bf16, tag="WsT")
            nc.sync.dma_start(out=WsT[:], in_=WsT_d[:])
            sgs = vec(sgs_d, 2, "sgs", f32r)
            cs = vec(cs_d, 2, "cs", f32)

            fmb = {}  # (b, m) -> [128, 1024] bf16, persists into small phase

            xst_all = {}

            def fetch_xs(c, b):
                t = sp.tile([128, 2 * CHUNK], bf16, tag="xs", bufs=8,
                            name=f"xs{c}{b}")
                nc.sync.dma_start(
                    out=t[:].rearrange("p (k x) -> p k x", k=2),
                    in_=xs_d[b, :, :, c * CHUNK:(c + 1) * CHUNK])
                xst_all[(c, b)] = t

            fetch_xs(0, 0)
            WcmT = wp.tile([128, 4 * 256], bf16, tag="WcmT")
            nc.sync.dma_start(out=WcmT[:], in_=WcmT_d[:])
            wfcm = vec(wfcm_d, 2, "wfcm", f32)
            bout = vec(bout_d, 2, "bout", f32)
            nc.sync.dma_start(out=xmt[1][:], in_=xm_d[1])
            WcsT = wp.tile([128, 2 * 256], bf16, tag="WcsT")
            nc.sync.dma_start(out=WcsT[:], in_=WcsT_d[:])
            wfcs = wp.tile([1, 256], bf16, tag="wfcs")
            nc.sync.dma_start(out=wfcs[:], in_=wfcs_d[:])
            fetch_xs(0, 1)

            # ================= main branches (32x32) =================
            for b in range(BPC):
                xm = xmt[b]
                t2m = [mp.tile([128, NPX_M], f32r, tag=f"tm{m}", bufs=2,
                               name=f"tm{m}_{b}") for m in range(4)]
                smf = mp.tile([1, NPX_M], f32, tag="smf", bufs=2)
                for n in range(2):
                    for m in range(4):
                        py = ps_y.tile([128, 512], f32, tag="y")
                        for k in range(4):
                            nc.tensor.matmul(
                                py[:], WmT[:, k * 512 + m * 128:k * 512 + (m + 1) * 128],
                                xm[:, k * NPX_M + n * 512:k * NPX_M + (n + 1) * 512],
                                start=(k == 0), stop=(k == 3))
                        nc.scalar.activation(t2m[m][:, n * 512:(n + 1) * 512], py[:],
                                             Act.Square, bias=cm[:, m:m + 1], scale=1.0)
                    pscr = ps_s.tile([1, 512], f32, tag="s")
                    for k in range(4):
                        nc.tensor.matmul(pscr[:], sgm[:, k:k + 1],
                                         t2m[k][:, n * 512:(n + 1) * 512],
                                         start=(k == 0), stop=(k == 3))
                    nc.scalar.activation(smf[:, n * 512:(n + 1) * 512], pscr[:], Act.Copy)

                # softmax over w (32-wide rows, natural px order at 32x32)
                shw = mp.tile([32, 32], f32, tag="shw", bufs=2)
                nc.sync.dma_start(out=shw[:], in_=smf[:])
                nmax = mp.tile([32, 1], f32, tag="nmax", bufs=2)
                nc.vector.tensor_reduce(nmax[:], shw[:], axis=mybir.AxisListType.X,
                                        op=Alu.max, negate=True)
                ex = mp.tile([32, 32], f32, tag="ex", bufs=2)
                esum = mp.tile([32, 1], f32, tag="esum", bufs=2)
                nc.scalar.activation(ex[:], shw[:], Act.Exp, bias=nmax[:], scale=1.0,
                                     accum_out=esum[:])
                rec = mp.tile([32, 1], f32, tag="rec", bufs=2)
                nc.vector.reciprocal(rec[:], esum[:])
                attnm = mp.tile([32, 32], bf16, tag="attnm", bufs=2)
                nc.scalar.activation(attnm[:], ex[:], Act.Copy, scale=rec[:])
                amf = mp.tile([1, NPX_M], bf16, tag="amf", bufs=2)
                nc.sync.dma_start(out=amf[:], in_=attnm[:])
                amb = mp.tile([128, NPX_M], bf16, tag="amb", bufs=2)
                nc.gpsimd.partition_broadcast(amb[:], amf[:])

                # x'm = xm * attn (bf16 2x TT); fused GEMM; fmb = amb*wfcm + pf
                xpm = mp.tile([128, 4 * NPX_M], bf16, tag="xpm", bufs=2)
                for k in range(4):
                    for n in range(2):
                        nc.vector.tensor_tensor(
                            xpm[:, k * NPX_M + n * 512:k * NPX_M + (n + 1) * 512],
                            xm[:, k * NPX_M + n * 512:k * NPX_M + (n + 1) * 512],
                            amb[:, n * 512:(n + 1) * 512], op=Alu.mult)
                for m in range(2):
                    fmb[(b, m)] = mp.tile([128, NPX_M], bf16, tag=f"fmb{b}{m}", bufs=1,
                                          name=f"fmb{b}{m}")
                for n in range(2):
                    for m in range(2):
                        pf = ps_f.tile([128, 512], f32, tag="f")
                        for k in range(4):
                            nc.tensor.matmul(
                                pf[:], WcmT[:, k * 256 + m * 128:k * 256 + (m + 1) * 128],
                                xpm[:, k * NPX_M + n * 512:k * NPX_M + (n + 1) * 512],
                                start=(k == 0), stop=(k == 3))
                        nc.vector.scalar_tensor_tensor(
                            fmb[(b, m)][:, n * 512:(n + 1) * 512],
                            amb[:, n * 512:(n + 1) * 512], wfcm[:, m:m + 1], pf[:],
                            op0=Alu.mult, op1=Alu.add)

            # ====== small branches; px' = (dh, h2l, dw, wj) within each chunk ======
            out_dmas = []
            pending_combines = []
            for c in range(NCHUNK):
                for b in range(BPC):
                    if (c, b) not in xst_all:
                        fetch_xs(c, b)
                    xst = xst_all[(c, b)]
                    if c + 1 < NCHUNK and (c + 1, b) not in xst_all:
                        fetch_xs(c + 1, b)

                    t2s = [sp.tile([128, CHUNK], f32r, tag=f"ts{m}", bufs=2,
                                   name=f"ts{m}") for m in range(2)]
                    sf = sp.tile([1, CHUNK], f32, tag="sf", bufs=2)
                    for n in range(2):
                        for m in range(2):
                            py = ps_y.tile([128, 512], f32, tag="y")
                            for k in range(2):
                                nc.tensor.matmul(
                                    py[:], WsT[:, k * 256 + m * 128:k * 256 + (m + 1) * 128],
                                    xst[:, k * CHUNK + n * 512:k * CHUNK + (n + 1) * 512],
                                    start=(k == 0), stop=(k == 1))
                            nc.scalar.activation(t2s[m][:, n * 512:(n + 1) * 512], py[:],
                                                 Act.Square, bias=cs[:, m:m + 1], scale=1.0)
                        pscr = ps_s.tile([1, 512], f32, tag="s")
                        for k in range(2):
                            nc.tensor.matmul(pscr[:], sgs[:, k:k + 1],
                                             t2s[k][:, n * 512:(n + 1) * 512],
                                             start=(k == 0), stop=(k == 1))
                        nc.scalar.activation(sf[:, n * 512:(n + 1) * 512], pscr[:], Act.Copy)

                    # softmax: rows (dh,h2l), cols (dw,wj); flat DMAs suffice
                    shw_s = sp.tile([16, 64], f32, tag="shw_s", bufs=2)
                    nc.sync.dma_start(out=shw_s[:], in_=sf[:])
                    nmax_s = sp.tile([16, 1], f32, tag="nmax_s", bufs=2)
                    nc.vector.tensor_reduce(nmax_s[:], shw_s[:], axis=mybir.AxisListType.X,
                                            op=Alu.max, negate=True)
                    ex_s = sp.tile([16, 64], f32, tag="ex_s", bufs=2)
                    esum_s = sp.tile([16, 1], f32, tag="esum_s", bufs=2)
                    nc.scalar.activation(ex_s[:], shw_s[:], Act.Exp, bias=nmax_s[:],
                                         scale=1.0, accum_out=esum_s[:])
                    rec_s = sp.tile([16, 1], f32, tag="rec_s", bufs=2)
                    nc.vector.reciprocal(rec_s[:], esum_s[:])
                    attn_s = sp.tile([16, 64], bf16, tag="attn_s", bufs=2)
                    nc.scalar.activation(attn_s[:], ex_s[:], Act.Copy, scale=rec_s[:])
                    af = sp.tile([1, CHUNK], bf16, tag="af", bufs=2)
                    nc.sync.dma_start(out=af[:], in_=attn_s[:])
                    asb = sp.tile([128, CHUNK], bf16, tag="asb", bufs=2)
                    nc.gpsimd.partition_broadcast(asb[:], af[:])

                    # x's = x * attn (bf16 2x TT)
                    xps = sp.tile([128, 2 * CHUNK], bf16, tag="xps", bufs=2)
                    for k in range(2):
                        for n in range(2):
                            nc.vector.tensor_tensor(
                                xps[:, k * CHUNK + n * 512:k * CHUNK + (n + 1) * 512],
                                xst[:, k * CHUNK + n * 512:k * CHUNK + (n + 1) * 512],
                                asb[:, n * 512:(n + 1) * 512], op=Alu.mult)

                    # fused GEMMs now; the combine stt's are deferred until
                    # after the NEXT chunk's x' TTs so the in-order DVE queue
                    # never blocks x' behind a combine that waits on PE
                    outc = sp.tile([128, 2 * CHUNK], bf16, tag="outc", bufs=3)
                    pfs = {}
                    for n in range(2):
                        for m in range(2):
                            pf = ps_f.tile([128, 512], f32, tag="f")
                            for k in range(2):
                                nc.tensor.matmul(
                                    pf[:], WcsT[:, k * 256 + m * 128:k * 256 + (m + 1) * 128],
                                    xps[:, k * CHUNK + n * 512:k * CHUNK + (n + 1) * 512],
                                    start=(k == 0), stop=False)
                            nc.tensor.matmul(
                                pf[:], wfcs[0:1, m * 128:(m + 1) * 128],
                                af[0:1, n * 512:(n + 1) * 512],
                                start=False, stop=True)
                            pfs[(n, m)] = pf

                    def emit_combine(outc=outc, pfs=pfs, c=c, b=b):
                        for n in range(2):
                            for m in range(2):
                                pf = pfs[(n, m)]
                                ov = outc[:, m * CHUNK + n * 512:m * CHUNK + (n + 1) * 512] \
                                    .rearrange("p (h dw w) -> p h dw w", h=8, dw=2)
                                pv3 = pf[:].rearrange("p (h dw w) -> p h dw w", h=8, dw=2)
                                fv = fmb[(b, m)][:, c * 256:(c + 1) * 256] \
                                    .rearrange("p (h w) -> p h w", h=8)
                                for dw in range(2):
                                    nc.vector.scalar_tensor_tensor(
                                        ov[:, :, dw], pv3[:, :, dw], bout[:, m:m + 1],
                                        fv[:], op0=Alu.add, op1=Alu.add)
                        out_dmas.append((outc, c, b))

                    pending_combines.append(emit_combine)
                    if len(pending_combines) > 1:
                        pending_combines.pop(0)()

            for fn in pending_combines:
                fn()

            for outc, c, b in out_dmas:
                for m in range(2):
                    nc.sync.dma_start(
                        out=out_d[b, :, m, c * CHUNK:(c + 1) * CHUNK],
                        in_=outc[:, m * CHUNK:(m + 1) * CHUNK])

    nc.compile()
    return nc


def _prep_weights(W_align, b_align, pos_embed_main, pos_embed_small,
                  W_qkv_s, b_qkv_s, W_proj_s, b_proj_s, rel_pos_s,
                  W_qkv_m, b_qkv_m, W_proj_m, b_proj_m, rel_pos_m,
                  W_fuse, b_fuse):
    d = np.float64
    W_align, b_align = W_align.astype(d), b_align.astype(d)
    pos_s = pos_embed_small.reshape(-1).astype(d)
    pos_m = pos_embed_main.reshape(-1).astype(d)
    rel_s = rel_pos_s.reshape(-1).astype(d)
    rel_m = rel_pos_m.reshape(-1).astype(d)
    W_qkv_s, b_qkv_s = W_qkv_s.astype(d), b_qkv_s.astype(d)
    W_qkv_m, b_qkv_m = W_qkv_m.astype(d), b_qkv_m.astype(d)
    W_proj_s, b_proj_s = W_proj_s.astype(d), b_proj_s.astype(d)
    W_proj_m, b_proj_m = W_proj_m.astype(d), b_proj_m.astype(d)
    W_fuse, b_fuse = W_fuse.astype(d), b_fuse.astype(d)
    scale = 1.0 / np.sqrt(np.float64(CS))

    def eig_form(A, u):
        n = A.shape[0]
        Bt = np.zeros((n + 1, n + 1))
        Bt[:n, :n] = (A + A.T) / 2
        Bt[:n, n] = u / 2
        Bt[n, :n] = u / 2
        lam, Q = np.linalg.eigh(Bt)
        keep = np.argsort(-np.abs(lam))[:n]
        lam, Q = lam[keep], Q[:, keep]
        Wt = np.sqrt(np.abs(lam))[:, None] * Q.T
        return Wt[:, :n], np.sign(lam), Wt[:, n]

    # small branch
    Wq, Wk, Wv = W_qkv_s[:CS], W_qkv_s[CS:2 * CS], W_qkv_s[2 * CS:]
    bq, bk, bv = b_qkv_s[:CS], b_qkv_s[CS:2 * CS], b_qkv_s[2 * CS:]
    cq = Wq @ pos_s + bq + rel_s
    ck = Wk @ pos_s + bk + rel_s
    cv_s = Wv @ pos_s + bv
    A_s = (Wq.T @ Wk) * scale
    u_s = (Wk.T @ cq + Wq.T @ ck) * scale
    Ws, sg_s, c_s = eig_form(A_s, u_s)

    # main branch (W_align folded; runs at 32x32)
    cbase = b_align + pos_m
    Wqm = W_qkv_m[:CS] @ W_align
    Wkm = W_qkv_m[CS:2 * CS] @ W_align
    Wvm = W_qkv_m[2 * CS:] @ W_align
    cqm = W_qkv_m[:CS] @ cbase + b_qkv_m[:CS] + rel_m
    ckm = W_qkv_m[CS:2 * CS] @ cbase + b_qkv_m[CS:2 * CS] + rel_m
    cv_m = W_qkv_m[2 * CS:] @ cbase + b_qkv_m[2 * CS:]
    A_m = (Wqm.T @ Wkm) * scale
    u_m = (Wkm.T @ cqm + Wqm.T @ ckm) * scale
    Wm, sg_m, c_m = eig_form(A_m, u_m)

    # fuse folding (1/2 on main: upsampled softmax over 64 = low-res softmax/2)
    W_fs = W_fuse[:, :CS] @ W_proj_s
    W_fm = (W_fuse[:, CS:] @ W_proj_m) * 0.5
    b_out = b_fuse + W_fuse[:, :CS] @ b_proj_s + W_fuse[:, CS:] @ b_proj_m

    # fused attn-through-proj GEMMs + rank-1 constants
    W_cs = W_fs @ Wv            # [256, 256]
    wfc_s = W_fs @ cv_s         # [256]
    W_cm = W_fm @ Wvm           # [256, 512]
    wfc_m = W_fm @ cv_m         # [256]

    f = np.float32

    def packT(W, kblocks):
        rows, ch = W.shape
        assert ch == kblocks * 128
        t = W.T.reshape(kblocks, 128, rows).transpose(1, 0, 2).reshape(128, kblocks * rows)
        return np.ascontiguousarray(t.astype(BF16))

    def colvec(v, chunks):
        return np.ascontiguousarray(v.reshape(chunks, 128).T.astype(f))

    return {
        "WmT": packT(Wm, 4), "sgm": colvec(sg_m, 4), "cm": colvec(c_m, 4),
        "WcmT": packT(W_cm, 4), "wfcm": colvec(wfc_m, 2), "bout": colvec(b_out, 2),
        "WsT": packT(Ws, 2), "sgs": colvec(sg_s, 2), "cs": colvec(c_s, 2),
        "WcsT": packT(W_cs, 2),
        "wfcs": np.ascontiguousarray(wfc_s.reshape(1, 256).astype(BF16)),
    }


def kernel(**inputs):
    global _prog
    from concourse.bass_utils import run_bass_kernel_spmd

    small = np.asarray(inputs["small_feat"], dtype=np.float32).reshape(B, CS, NPX_S)
    main = np.asarray(inputs["main_feat"], dtype=np.float32).reshape(B, CM, NPX_M)
    w = _prep_weights(**{k: np.asarray(v) for k, v in inputs.items()
                         if k not in ("small_feat", "main_feat")})

    small_p = small[:, :, PERM].astype(BF16)                       # [B, 256, 4096]
    xs_pack = np.ascontiguousarray(
        small_p.reshape(B, 2, 128, NPX_S).transpose(0, 2, 1, 3))   # [B, 128, 2, 4096]
    xm_pack = np.ascontiguousarray(
        main.astype(BF16).reshape(B, 4, 128, NPX_M)
        .transpose(0, 2, 1, 3).reshape(B, 128, 4 * NPX_M))         # [B, 128, 4096]

    if _prog is None:
        _prog = _build_program()
    nc = _prog

    in_maps = []
    for c in range(NCORES):
        m = dict(w)
        m["xs"] = xs_pack[c * BPC:(c + 1) * BPC]
        m["xm"] = xm_pack[c * BPC:(c + 1) * BPC]
        in_maps.append(m)

    res = run_bass_kernel_spmd(nc, in_maps, list(range(NCORES)))
    out_p = np.concatenate([r["out"] for r in res.results], axis=0)  # [B,128,2,4096]
    out_p = out_p.astype(np.float32).transpose(0, 2, 1, 3).reshape(B, CS, NPX_S)
    out = np.empty((B, CS, NPX_S), np.float32)
    out[:, :, PERM] = out_p
    return out.reshape(B, CS, HS, WS)


# revision 39
# speedup vs baseline: 1.0239x; 1.0239x over previous
"""CrossScaleAttention Trainium2 kernel (v3).

Data-parallel over batch: 16 samples / 8 cores = 2 samples per core.

Algebraic restructuring (exact up to fp rounding):
  - 1x1 convs commute with nearest 2x upsample -> main branch runs at 32x32.
  - W_align folded into main qkv; pos/rel/biases folded into constants.
  - attention score via AUGMENTED EIGENDECOMPOSITION:
      score = x^T A x + u.x = [x;1]^T Btil [x;1],  Btil symmetric (C+1)^2.
      eigh(Btil), keep C largest-|lam| comps (drops ~1e-6 residual):
      score = sum_i sgn_i * (W_i.x + c_i)^2 + const(dropped by softmax).
    Square runs on the Activation engine with c as per-partition bias.
  - attn scales pixels (columns), so it commutes through the proj GEMM:
      Wf @ ((Wv x + cv) * attn) = (Wf Wv) @ (x * attn) + (Wf cv) (x) attn
    One fused GEMM (Wc = Wf Wv) on x' = x*attn (a 2x-rate bf16 DVE multiply)
    plus a rank-1 matmul (lhsT = Wf cv, rhs = attn row) into the same PSUM.
  - bout folded into the small-branch combine stt; main fmb = (amb*wfcm)+pf.
  - small-branch pixels host-permuted to (dh, h2l, dw, wj) so the softmax
    transpose DMAs are flat copies and the 2x2 upsample-add segments are
    contiguous-innermost views of PSUM / fmb.
  - bf16 inputs/weights/attn/outputs; f32r square/score path (t' needs f32
    mantissa); matmuls accumulate in f32 PSUM.
  - out-DMAs emitted after all input DMAs on the in-order SP queue so their
    waits never block input prefetch.
"""
import sys
sys.path.insert(0, '/opt/trn_rl_repo')
import numpy as np
import ml_dtypes

B, CS, CM = 16, 256, 512
HS = WS = 64
HM = WM = 32
NPX_S = HS * WS          # 4096
NPX_M = HM * WM          # 1024
NCORES = 8
BPC = B // NCORES        # 2 samples per core
CHUNK = 1024             # small-branch pixel chunk (16 h-rows)
NCHUNK = NPX_S // CHUNK  # 4

BF16 = ml_dtypes.bfloat16


# pixel permutation within a chunk: px' = (dh, h2l, dw, wj), px = h*64+w with
# h = c*16 + 2*h2l + dh, w = 2*wj + dw  (c = chunk index)
def _chunk_perm():
    dh, h2l, dw, wj = np.meshgrid(np.arange(2), np.arange(8), np.arange(2),
                                  np.arange(32), indexing='ij')
    h = 2 * h2l + dh
    w = 2 * wj + dw
    return (h * 64 + w).reshape(-1)


_PERM_CHUNK = _chunk_perm()
PERM = np.concatenate([c * CHUNK + _PERM_CHUNK for c in range(NCHUNK)])  # [4096]

_prog = None


def _build_program():
    import concourse.bacc as bacc
    import concourse.mybir as mybir
    from concourse.tile import TileContext

    f32, f32r, bf16 = mybir.dt.float32, mybir.dt.float32r, mybir.dt.bfloat16
    Act = mybir.ActivationFunctionType
    Alu = mybir.AluOpType

    nc = bacc.Bacc(None, target_bir_lowering=False)

    xs_d = nc.dram_tensor("xs", [BPC, 128, 2, NPX_S], bf16, kind="ExternalInput")
    xm_d = nc.dram_tensor("xm", [BPC, 128, 4 * NPX_M], bf16, kind="ExternalInput")
    WmT_d = nc.dram_tensor("WmT", [128, 4 * 512], bf16, kind="ExternalInput")
    WcmT_d = nc.dram_tensor("WcmT", [128, 4 * 256], bf16, kind="ExternalInput")
    WsT_d = nc.dram_tensor("WsT", [128, 2 * 256], bf16, kind="ExternalInput")
    WcsT_d = nc.dram_tensor("WcsT", [128, 2 * 256], bf16, kind="ExternalInput")
    sgm_d = nc.dram_tensor("sgm", [128, 4], f32r, kind="ExternalInput")
    cm_d = nc.dram_tensor("cm", [128, 4], f32, kind="ExternalInput")
    wfcm_d = nc.dram_tensor("wfcm", [128, 2], f32, kind="ExternalInput")
    bout_d = nc.dram_tensor("bout", [128, 2], f32, kind="ExternalInput")
    sgs_d = nc.dram_tensor("sgs", [128, 2], f32r, kind="ExternalInput")
    cs_d = nc.dram_tensor("cs", [128, 2], f32, kind="ExternalInput")
    wfcs_d = nc.dram_tensor("wfcs", [1, 256], bf16, kind="ExternalInput")
    out_d = nc.dram_tensor("out", [BPC, 128, 2, NPX_S], bf16, kind="ExternalOutput")

    with TileContext(nc) as tc:
        with (
            tc.tile_pool(name="wp", bufs=1) as wp,
            tc.tile_pool(name="mp", bufs=1) as mp,
            tc.tile_pool(name="sp", bufs=1) as sp,
            tc.tile_pool(name="ps_y", bufs=3, space="PSUM") as ps_y,
            tc.tile_pool(name="ps_f", bufs=3, space="PSUM") as ps_f,
            tc.tile_pool(name="ps_s", bufs=2, space="PSUM") as ps_s,
        ):
            # ---- resident weights; k-interleaved so the first matmul can
            # start after ~380KB instead of the full main working set
            WmT = wp.tile([128, 4 * 512], bf16, tag="WmT")
            xmt = []
            for b in range(BPC):
                t = mp.tile([128, 4 * NPX_M], bf16, tag="xm", bufs=2, name=f"xm{b}")
                xmt.append(t)
            for k in range(4):
                nc.sync.dma_start(out=WmT[:, k * 512:(k + 1) * 512],
                                  in_=WmT_d[:, k * 512:(k + 1) * 512])
                nc.sync.dma_start(out=xmt[0][:, k * NPX_M:(k + 1) * NPX_M],
                                  in_=xm_d[0, :, k * NPX_M:(k + 1) * NPX_M])

            def vec(dram, cols, name, dt):
                t = wp.tile([128, cols], dt, tag=name)
                nc.sync.dma_start(out=t[:], in_=dram[:])
                return t

            sgm = vec(sgm_d, 4, "sgm", f32r)
            cm = vec(cm_d, 4, "cm", f32)
            # small-branch score weights early: lets the scheduler fill main-
            # phase PE bubbles with chunk-0 GEMMs
            WsT = wp.tile([128, 2 * 256], bf16, tag="WsT")
            nc.sync.dma_start(out=WsT[:], in_=WsT_d[:])
            sgs = vec(sgs_d, 2, "sgs", f32r)
            cs = vec(cs_d, 2, "cs", f32)

            fmb = {}  # (b, m) -> [128, 1024] bf16, persists into small phase

            xst_all = {}

            def fetch_xs(c, b):
                t = sp.tile([128, 2 * CHUNK], bf16, tag="xs", bufs=8,
                            name=f"xs{c}{b}")
                nc.sync.dma_start(
                    out=t[:].rearrange("p (k x) -> p k x", k=2),
                    in_=xs_d[b, :, :, c * CHUNK:(c + 1) * CHUNK])
                xst_all[(c, b)] = t

            fetch_xs(0, 0)
            WcmT = wp.tile([128, 4 * 256], bf16, tag="WcmT")
            nc.sync.dma_start(out=WcmT[:], in_=WcmT_d[:])
            wfcm = vec(wfcm_d, 2, "wfcm", f32)
            bout = vec(bout_d, 2, "bout", f32)
            nc.sync.dma_start(out=xmt[1][:], in_=xm_d[1])
            WcsT = wp.tile([128, 2 * 256], bf16, tag="WcsT")
            nc.sync.dma_start(out=WcsT[:], in_=WcsT_d[:])
            wfcs = wp.tile([1, 256], bf16, tag="wfcs")
            nc.sync.dma_start(out=wfcs[:], in_=wfcs_d[:])
            fetch_xs(0, 1)

            # ================= main branches (32x32), b0/b1 stage-skewed =====
            def main_a(b):
                xm = xmt[b]
                t2m = [mp.tile([128, NPX_M], f32r, tag=f"tm{m}", bufs=2,
                               name=f"tm{m}_{b}") for m in range(4)]
                smf = mp.tile([1, NPX_M], f32, tag="smf", bufs=2)
                for n in range(2):
                    for m in range(4):
                        py = ps_y.tile([128, 512], f32, tag="y")
                        for k in range(4):
                            nc.tensor.matmul(
                                py[:], WmT[:, k * 512 + m * 128:k * 512 + (m + 1) * 128],
                                xm[:, k * NPX_M + n * 512:k * NPX_M + (n + 1) * 512],
                                start=(k == 0), stop=(k == 3))
                        nc.scalar.activation(t2m[m][:, n * 512:(n + 1) * 512], py[:],
                                             Act.Square, bias=cm[:, m:m + 1], scale=1.0)
                    pscr = ps_s.tile([1, 512], f32, tag="s")
                    for k in range(4):
                        nc.tensor.matmul(pscr[:], sgm[:, k:k + 1],
                                         t2m[k][:, n * 512:(n + 1) * 512],
                                         start=(k == 0), stop=(k == 3))
                    nc.scalar.activation(smf[:, n * 512:(n + 1) * 512], pscr[:], Act.Copy)
                return smf

            def main_bc(b, smf):
                xm = xmt[b]
                # softmax over w (32-wide rows, natural px order at 32x32)
                shw = mp.tile([32, 32], f32, tag="shw", bufs=2)
                nc.sync.dma_start(out=shw[:], in_=smf[:])
                nmax = mp.tile([32, 1], f32, tag="nmax", bufs=2)
                nc.vector.tensor_reduce(nmax[:], shw[:], axis=mybir.AxisListType.X,
                                        op=Alu.max, negate=True)
                ex = mp.tile([32, 32], f32, tag="ex", bufs=2)
                esum = mp.tile([32, 1], f32, tag="esum", bufs=2)
                nc.scalar.activation(ex[:], shw[:], Act.Exp, bias=nmax[:], scale=1.0,
                                     accum_out=esum[:])
                rec = mp.tile([32, 1], f32, tag="rec", bufs=2)
                nc.vector.reciprocal(rec[:], esum[:])
                attnm = mp.tile([32, 32], bf16, tag="attnm", bufs=2)
                nc.scalar.activation(attnm[:], ex[:], Act.Copy, scale=rec[:])
                amf = mp.tile([1, NPX_M], bf16, tag="amf", bufs=2)
                nc.sync.dma_start(out=amf[:], in_=attnm[:])
                amb = mp.tile([128, NPX_M], bf16, tag="amb", bufs=2)
                nc.gpsimd.partition_broadcast(amb[:], amf[:])

                # x'm = xm * attn (bf16 2x TT); fused GEMM; fmb = amb*wfcm + pf
                xpm = mp.tile([128, 4 * NPX_M], bf16, tag="xpm", bufs=2)
                for k in range(4):
                    for n in range(2):
                        nc.vector.tensor_tensor(
                            xpm[:, k * NPX_M + n * 512:k * NPX_M + (n + 1) * 512],
                            xm[:, k * NPX_M + n * 512:k * NPX_M + (n + 1) * 512],
                            amb[:, n * 512:(n + 1) * 512], op=Alu.mult)
                for m in range(2):
                    fmb[(b, m)] = mp.tile([128, NPX_M], bf16, tag=f"fmb{b}{m}", bufs=1,
                                          name=f"fmb{b}{m}")
                for n in range(2):
                    for m in range(2):
                        pf = ps_f.tile([128, 512], f32, tag="f")
                        for k in range(4):
                            nc.tensor.matmul(
                                pf[:], WcmT[:, k * 256 + m * 128:k * 256 + (m + 1) * 128],
                                xpm[:, k * NPX_M + n * 512:k * NPX_M + (n + 1) * 512],
                                start=(k == 0), stop=(k == 3))
                        nc.vector.scalar_tensor_tensor(
                            fmb[(b, m)][:, n * 512:(n + 1) * 512],
                            amb[:, n * 512:(n + 1) * 512], wfcm[:, m:m + 1], pf[:],
                            op0=Alu.mult, op1=Alu.add)

            smf0 = main_a(0)
            smf1 = main_a(1)

            # ====== small branches; px' = (dh, h2l, dw, wj) within each chunk ======
            # Software-pipelined emission with a one-unit skew: stage A
            # (score GEMMs+Squares) of unit u is emitted before stage B
            # (softmax) and C (fused GEMMs) of unit u-1, so each in-order
            # engine queue always has ready work ahead of chain-latency ops.
            out_dmas = []
            pending_combines = []
            units = [(c, b) for c in range(NCHUNK) for b in range(BPC)]

            def stage_a(u):
                c, b = u
                if (c, b) not in xst_all:
                    fetch_xs(c, b)
                xst = xst_all[(c, b)]
                if c + 1 < NCHUNK and (c + 1, b) not in xst_all:
                    fetch_xs(c + 1, b)
                t2s = [sp.tile([128, CHUNK], f32r, tag=f"ts{m}", bufs=3,
                               name=f"ts{m}") for m in range(2)]
                sf = sp.tile([1, CHUNK], f32, tag="sf", bufs=3)
                for n in range(2):
                    for m in range(2):
                        py = ps_y.tile([128, 512], f32, tag="y")
                        for k in range(2):
                            nc.tensor.matmul(
                                py[:], WsT[:, k * 256 + m * 128:k * 256 + (m + 1) * 128],
                                xst[:, k * CHUNK + n * 512:k * CHUNK + (n + 1) * 512],
                                start=(k == 0), stop=(k == 1))
                        nc.scalar.activation(t2s[m][:, n * 512:(n + 1) * 512], py[:],
                                             Act.Square, bias=cs[:, m:m + 1], scale=1.0)
                    pscr = ps_s.tile([1, 512], f32, tag="s")
                    for k in range(2):
                        nc.tensor.matmul(pscr[:], sgs[:, k:k + 1],
                                         t2s[k][:, n * 512:(n + 1) * 512],
                                         start=(k == 0), stop=(k == 1))
                    nc.scalar.activation(sf[:, n * 512:(n + 1) * 512], pscr[:], Act.Copy)
                return {"xst": xst, "sf": sf}

            def stage_b(st):
                # softmax: rows (dh,h2l), cols (dw,wj). Each dh-half (8 rows
                # = one n-block) is independent, so the whole chain down to
                # the broadcast runs per-half and stage C's n0 GEMMs start a
                # broadcast-latency earlier.
                # halves live side-by-side on partitions 0-7 (engine ops
                # must start at partition 0)
                shw_s = sp.tile([8, 2 * 64], f32, tag="shw_s", bufs=4)
                nmax_s = sp.tile([8, 2], f32, tag="nmax_s", bufs=4)
                ex_s = sp.tile([8, 2 * 64], f32, tag="ex_s", bufs=4)
                esum_s = sp.tile([8, 2], f32, tag="esum_s", bufs=4)
                rec_s = sp.tile([8, 2], f32, tag="rec_s", bufs=4)
                attn_s = sp.tile([8, 2 * 64], bf16, tag="attn_s", bufs=4)
                af = sp.tile([1, CHUNK], bf16, tag="af", bufs=4)
                asb = sp.tile([128, CHUNK], bf16, tag="asb", bufs=2)
                for n in range(2):
                    ws = slice(n * 64, (n + 1) * 64)
                    cseg = slice(n * 512, (n + 1) * 512)
                    nc.sync.dma_start(out=shw_s[:, ws], in_=st["sf"][:, cseg])
                    nc.vector.tensor_reduce(nmax_s[:, n:n + 1], shw_s[:, ws],
                                            axis=mybir.AxisListType.X,
                                            op=Alu.max, negate=True)
                    nc.scalar.activation(ex_s[:, ws], shw_s[:, ws], Act.Exp,
                                         bias=nmax_s[:, n:n + 1], scale=1.0,
                                         accum_out=esum_s[:, n:n + 1])
                    nc.vector.reciprocal(rec_s[:, n:n + 1], esum_s[:, n:n + 1])
                    nc.scalar.activation(attn_s[:, ws], ex_s[:, ws], Act.Copy,
                                         scale=rec_s[:, n:n + 1])
                    nc.sync.dma_start(out=af[:, cseg], in_=attn_s[:, ws])
                    nc.gpsimd.partition_broadcast(asb[:, cseg], af[:, cseg])
                st["af"], st["asb"] = af, asb

            def stage_c(st, u):
                c, b = u
                xst, af, asb = st["xst"], st["af"], st["asb"]
                # x's = x * attn (bf16 2x TT)
                xps = sp.tile([128, 2 * CHUNK], bf16, tag="xps", bufs=3)
                for n in range(2):
                    for k in range(2):
                        nc.vector.tensor_tensor(
                            xps[:, k * CHUNK + n * 512:k * CHUNK + (n + 1) * 512],
                            xst[:, k * CHUNK + n * 512:k * CHUNK + (n + 1) * 512],
                            asb[:, n * 512:(n + 1) * 512], op=Alu.mult)
                # out = Wcs@x' + wfcs(x)attn (+bout +fmb_up in the combine)
                outc = sp.tile([128, 2 * CHUNK], bf16, tag="outc", bufs=3)
                pfs = {}
                for n in range(2):
                    for m in range(2):
                        pf = ps_f.tile([128, 512], f32, tag="f")
                        for k in range(2):
                            nc.tensor.matmul(
                                pf[:], WcsT[:, k * 256 + m * 128:k * 256 + (m + 1) * 128],
                                xps[:, k * CHUNK + n * 512:k * CHUNK + (n + 1) * 512],
                                start=(k == 0), stop=False)
                        nc.tensor.matmul(
                            pf[:], wfcs[0:1, m * 128:(m + 1) * 128],
                            af[0:1, n * 512:(n + 1) * 512],
                            start=False, stop=True)
                        pfs[(n, m)] = pf

                def emit_combine(outc=outc, pfs=pfs, c=c, b=b):
                    for m in range(2):
                        for n in range(2):
                            pf = pfs[(n, m)]
                            ov = outc[:, m * CHUNK + n * 512:m * CHUNK + (n + 1) * 512] \
                                .rearrange("p (h dw w) -> p h dw w", h=8, dw=2)
                            pv3 = pf[:].rearrange("p (h dw w) -> p h dw w", h=8, dw=2)
                            fv = fmb[(b, m)][:, c * 256:(c + 1) * 256] \
                                .rearrange("p (h w) -> p h w", h=8)
                            for dw in range(2):
                                nc.vector.scalar_tensor_tensor(
                                    ov[:, :, dw], pv3[:, :, dw], bout[:, m:m + 1],
                                    fv[:], op0=Alu.add, op1=Alu.add)
                    out_dmas.append((outc, c, b))

                pending_combines.append(emit_combine)
                if len(pending_combines) > 1:
                    pending_combines.pop(0)()

            # prologue: interleave the first small A-stages with the main
            # softmax/value stages so PE never waits on a softmax chain;
            # steady state runs a two-unit skew (A two ahead of B/C)
            queue = []

            def drain_one():
                uu, ss = queue.pop(0)
                stage_b(ss)
                stage_c(ss, uu)

            main_bc(0, smf0)
            main_bc(1, smf1)
            for u in units:
                queue.append((u, stage_a(u)))
                if len(queue) > 1:
                    drain_one()
            while queue:
                drain_one()

            for fn in pending_combines:
                fn()

            for outc, c, b in out_dmas:
                for m in range(2):
                    nc.sync.dma_start(
                        out=out_d[b, :, m, c * CHUNK:(c + 1) * CHUNK],
                        in_=outc[:, m * CHUNK:(m + 1) * CHUNK])

    nc.compile()
    return nc


def _prep_weights(W_align, b_align, pos_embed_main, pos_embed_small,
                  W_qkv_s, b_qkv_s, W_proj_s, b_proj_s, rel_pos_s,
                  W_qkv_m, b_qkv_m, W_proj_m, b_proj_m, rel_pos_m,
                  W_fuse, b_fuse):
    d = np.float64
    W_align, b_align = W_align.astype(d), b_align.astype(d)
    pos_s = pos_embed_small.reshape(-1).astype(d)
    pos_m = pos_embed_main.reshape(-1).astype(d)
    rel_s = rel_pos_s.reshape(-1).astype(d)
    rel_m = rel_pos_m.reshape(-1).astype(d)
    W_qkv_s, b_qkv_s = W_qkv_s.astype(d), b_qkv_s.astype(d)
    W_qkv_m, b_qkv_m = W_qkv_m.astype(d), b_qkv_m.astype(d)
    W_proj_s, b_proj_s = W_proj_s.astype(d), b_proj_s.astype(d)
    W_proj_m, b_proj_m = W_proj_m.astype(d), b_proj_m.astype(d)
    W_fuse, b_fuse = W_fuse.astype(d), b_fuse.astype(d)
    scale = 1.0 / np.sqrt(np.float64(CS))

    def eig_form(A, u):
        n = A.shape[0]
        Bt = np.zeros((n + 1, n + 1))
        Bt[:n, :n] = (A + A.T) / 2
        Bt[:n, n] = u / 2
        Bt[n, :n] = u / 2
        lam, Q = np.linalg.eigh(Bt)
        keep = np.argsort(-np.abs(lam))[:n]
        lam, Q = lam[keep], Q[:, keep]
        Wt = np.sqrt(np.abs(lam))[:, None] * Q.T
        return Wt[:, :n], np.sign(lam), Wt[:, n]

    # small branch
    Wq, Wk, Wv = W_qkv_s[:CS], W_qkv_s[CS:2 * CS], W_qkv_s[2 * CS:]
    bq, bk, bv = b_qkv_s[:CS], b_qkv_s[CS:2 * CS], b_qkv_s[2 * CS:]
    cq = Wq @ pos_s + bq + rel_s
    ck = Wk @ pos_s + bk + rel_s
    cv_s = Wv @ pos_s + bv
    A_s = (Wq.T @ Wk) * scale
    u_s = (Wk.T @ cq + Wq.T @ ck) * scale
    Ws, sg_s, c_s = eig_form(A_s, u_s)

    # main branch (W_align folded; runs at 32x32)
    cbase = b_align + pos_m
    Wqm = W_qkv_m[:CS] @ W_align
    Wkm = W_qkv_m[CS:2 * CS] @ W_align
    Wvm = W_qkv_m[2 * CS:] @ W_align
    cqm = W_qkv_m[:CS] @ cbase + b_qkv_m[:CS] + rel_m
    ckm = W_qkv_m[CS:2 * CS] @ cbase + b_qkv_m[CS:2 * CS] + rel_m
    cv_m = W_qkv_m[2 * CS:] @ cbase + b_qkv_m[2 * CS:]
    A_m = (Wqm.T @ Wkm) * scale
    u_m = (Wkm.T @ cqm + Wqm.T @ ckm) * scale
    Wm, sg_m, c_m = eig_form(A_m, u_m)

    # fuse folding (1/2 on main: upsampled softmax over 64 = low-res softmax/2)
    W_fs = W_fuse[:, :CS] @ W_proj_s
    W_fm = (W_fuse[:, CS:] @ W_proj_m) * 0.5
    b_out = b_fuse + W_fuse[:, :CS] @ b_proj_s + W_fuse[:, CS:] @ b_proj_m

    # fused attn-through-proj GEMMs + rank-1 constants
    W_cs = W_fs @ Wv            # [256, 256]
    wfc_s = W_fs @ cv_s         # [256]
    W_cm = W_fm @ Wvm           # [256, 512]
    wfc_m = W_fm @ cv_m         # [256]

    f = np.float32

    def packT(W, kblocks):
        rows, ch = W.shape
        assert ch == kblocks * 128
        t = W.T.reshape(kblocks, 128, rows).transpose(1, 0, 2).reshape(128, kblocks * rows)
        return np.ascontiguousarray(t.astype(BF16))

    def colvec(v, chunks):
        return np.ascontiguousarray(v.reshape(chunks, 128).T.astype(f))

    return {
        "WmT": packT(Wm, 4), "sgm": colvec(sg_m, 4), "cm": colvec(c_m, 4),
        "WcmT": packT(W_cm, 4), "wfcm": colvec(wfc_m, 2), "bout": colvec(b_out, 2),
        "WsT": packT(Ws, 2), "sgs": colvec(sg_s, 2), "cs": colvec(c_s, 2),
        "WcsT": packT(W_cs, 2),
        "wfcs": np.ascontiguousarray(wfc_s.reshape(1, 256).astype(BF16)),
    }


def kernel(**inputs):
    global _prog
    from concourse.bass_utils import run_bass_kernel_spmd

    small = np.asarray(inputs["small_feat"], dtype=np.float32).reshape(B, CS, NPX_S)
    main = np.asarray(inputs["main_feat"], dtype=np.float32).reshape(B, CM, NPX_M)
    w = _prep_weights(**{k: np.asarray(v) for k, v in inputs.items()
                         if k not in ("small_feat", "main_feat")})

    small_p = small[:, :, PERM].astype(BF16)                       # [B, 256, 4096]
    xs_pack = np.ascontiguousarray(
        small_p.reshape(B, 2, 128, NPX_S).transpose(0, 2, 1, 3))   # [B, 128, 2, 4096]
    xm_pack = np.ascontiguousarray(
        main.astype(BF16).reshape(B, 4, 128, NPX_M)
        .transpose(0, 2, 1, 3).reshape(B, 128, 4 * NPX_M))         # [B, 128, 4096]

    if _prog is None:
        _prog = _build_program()
    nc = _prog

    in_maps = []
    for c in range(NCORES):
        m = dict(w)
        m["xs"] = xs_pack[c * BPC:(c + 1) * BPC]
        m["xm"] = xm_pack[c * BPC:(c + 1) * BPC]
        in_maps.append(m)

    res = run_bass_kernel_spmd(nc, in_maps, list(range(NCORES)))
    out_p = np.concatenate([r["out"] for r in res.results], axis=0)  # [B,128,2,4096]
    out_p = out_p.astype(np.float32).transpose(0, 2, 1, 3).reshape(B, CS, NPX_S)
    out = np.empty((B, CS, NPX_S), np.float32)
    out[:, :, PERM] = out_p
    return out.reshape(B, CS, HS, WS)


# revision 47
# speedup vs baseline: 1.0418x; 1.0174x over previous
"""CrossScaleAttention Trainium2 kernel (v3).

Data-parallel over batch: 16 samples / 8 cores = 2 samples per core.

Algebraic restructuring (exact up to fp rounding):
  - 1x1 convs commute with nearest 2x upsample -> main branch runs at 32x32.
  - W_align folded into main qkv; pos/rel/biases folded into constants.
  - attention score via AUGMENTED EIGENDECOMPOSITION:
      score = x^T A x + u.x = [x;1]^T Btil [x;1],  Btil symmetric (C+1)^2.
      eigh(Btil), keep C largest-|lam| comps (drops ~1e-6 residual):
      score = sum_i sgn_i * (W_i.x + c_i)^2 + const(dropped by softmax).
    Square runs on the Activation engine with c as per-partition bias.
  - attn scales pixels (columns), so it commutes through the proj GEMM:
      Wf @ ((Wv x + cv) * attn) = (Wf Wv) @ (x * attn) + (Wf cv) (x) attn
    One fused GEMM (Wc = Wf Wv) on x' = x*attn (a 2x-rate bf16 DVE multiply)
    plus a rank-1 matmul (lhsT = Wf cv, rhs = attn row) into the same PSUM.
  - bout folded into the small-branch combine stt; main fmb = (amb*wfcm)+pf.
  - small-branch pixels host-permuted to (dh, h2l, dw, wj) so the softmax
    transpose DMAs are flat copies and the 2x2 upsample-add segments are
    contiguous-innermost views of PSUM / fmb.
  - bf16 inputs/weights/attn/outputs; f32r square/score path (t' needs f32
    mantissa); matmuls accumulate in f32 PSUM.
  - out-DMAs emitted after all input DMAs on the in-order SP queue so their
    waits never block input prefetch.
"""
import sys
sys.path.insert(0, '/opt/trn_rl_repo')
import numpy as np
import ml_dtypes

B, CS, CM = 16, 256, 512
HS = WS = 64
HM = WM = 32
NPX_S = HS * WS          # 4096
NPX_M = HM * WM          # 1024
NCORES = 8
BPC = B // NCORES        # 2 samples per core
CHUNK = 1024             # small-branch pixel chunk (16 h-rows)
NCHUNK = NPX_S // CHUNK  # 4

BF16 = ml_dtypes.bfloat16


# pixel permutation within a chunk: px' = (dh, h2l, dw, wj), px = h*64+w with
# h = c*16 + 2*h2l + dh, w = 2*wj + dw  (c = chunk index)
def _chunk_perm():
    dh, h2l, dw, wj = np.meshgrid(np.arange(2), np.arange(8), np.arange(2),
                                  np.arange(32), indexing='ij')
    h = 2 * h2l + dh
    w = 2 * wj + dw
    return (h * 64 + w).reshape(-1)


_PERM_CHUNK = _chunk_perm()
PERM = np.concatenate([c * CHUNK + _PERM_CHUNK for c in range(NCHUNK)])  # [4096]

_prog = None


def _build_program():
    import concourse.bacc as bacc
    import concourse.mybir as mybir
    from concourse.tile import TileContext

    f32, f32r, bf16 = mybir.dt.float32, mybir.dt.float32r, mybir.dt.bfloat16
    Act = mybir.ActivationFunctionType
    Alu = mybir.AluOpType

    nc = bacc.Bacc(None, target_bir_lowering=False)

    xs_d = nc.dram_tensor("xs", [BPC, 128, 2, NPX_S], bf16, kind="ExternalInput")
    xm_d = nc.dram_tensor("xm", [BPC, 128, 4 * NPX_M], bf16, kind="ExternalInput")
    WalT_d = nc.dram_tensor("WalT", [128, 4 * 256], bf16, kind="ExternalInput")
    zbias_d = nc.dram_tensor("zbias", [128, 2], f32, kind="ExternalInput")
    WzT_d = nc.dram_tensor("WzT", [128, 2 * 256], bf16, kind="ExternalInput")
    WczT_d = nc.dram_tensor("WczT", [128, 2 * 256], bf16, kind="ExternalInput")
    WsT_d = nc.dram_tensor("WsT", [128, 2 * 256], bf16, kind="ExternalInput")
    WcsT_d = nc.dram_tensor("WcsT", [128, 2 * 256], bf16, kind="ExternalInput")
    sgm_d = nc.dram_tensor("sgm", [128, 2], f32r, kind="ExternalInput")
    cm_d = nc.dram_tensor("cm", [128, 2], f32, kind="ExternalInput")
    wfcm_d = nc.dram_tensor("wfcm", [128, 2], f32, kind="ExternalInput")
    bout_d = nc.dram_tensor("bout", [128, 2], f32, kind="ExternalInput")
    sgs_d = nc.dram_tensor("sgs", [128, 2], f32r, kind="ExternalInput")
    cs_d = nc.dram_tensor("cs", [128, 2], f32, kind="ExternalInput")
    wfcs_d = nc.dram_tensor("wfcs", [1, 256], bf16, kind="ExternalInput")
    out_d = nc.dram_tensor("out", [BPC, 128, 2, NPX_S], bf16, kind="ExternalOutput")

    with TileContext(nc) as tc:
        with (
            tc.tile_pool(name="wp", bufs=1) as wp,
            tc.tile_pool(name="mp", bufs=1) as mp,
            tc.tile_pool(name="sp", bufs=1) as sp,
            tc.tile_pool(name="ps_y", bufs=3, space="PSUM") as ps_y,
            tc.tile_pool(name="ps_f", bufs=3, space="PSUM") as ps_f,
            tc.tile_pool(name="ps_s", bufs=2, space="PSUM") as ps_s,
        ):
            # ---- resident weights; k-interleaved so the first matmul can
            # start after ~190KB instead of the full main working set
            WalT = wp.tile([128, 4 * 256], bf16, tag="WalT")
            xmt = []
            for b in range(BPC):
                t = mp.tile([128, 4 * NPX_M], bf16, tag="xm", bufs=2, name=f"xm{b}")
                xmt.append(t)
            for k in range(4):
                nc.sync.dma_start(out=WalT[:, k * 256:(k + 1) * 256],
                                  in_=WalT_d[:, k * 256:(k + 1) * 256])
                nc.sync.dma_start(out=xmt[0][:, k * NPX_M:(k + 1) * NPX_M],
                                  in_=xm_d[0, :, k * NPX_M:(k + 1) * NPX_M])

            def vec(dram, cols, name, dt):
                t = wp.tile([128, cols], dt, tag=name)
                nc.sync.dma_start(out=t[:], in_=dram[:])
                return t

            zbias = vec(zbias_d, 2, "zbias", f32)
            WzT = wp.tile([128, 2 * 256], bf16, tag="WzT")
            nc.sync.dma_start(out=WzT[:], in_=WzT_d[:])
            sgm = vec(sgm_d, 2, "sgm", f32r)
            cm = vec(cm_d, 2, "cm", f32)
            # small-branch score weights early: lets the scheduler fill main-
            # phase PE bubbles with chunk-0 GEMMs
            WsT = wp.tile([128, 2 * 256], bf16, tag="WsT")
            nc.sync.dma_start(out=WsT[:], in_=WsT_d[:])
            sgs = vec(sgs_d, 2, "sgs", f32r)
            cs = vec(cs_d, 2, "cs", f32)

            fmb = {}  # (b, m) -> [128, 1024] bf16, persists into small phase

            xst_all = {}

            def fetch_xs(c, b):
                t = sp.tile([128, 2 * CHUNK], bf16, tag="xs", bufs=8,
                            name=f"xs{c}{b}")
                nc.sync.dma_start(
                    out=t[:].rearrange("p (k x) -> p k x", k=2),
                    in_=xs_d[b, :, :, c * CHUNK:(c + 1) * CHUNK])
                xst_all[(c, b)] = t

            fetch_xs(0, 0)
            WczT = wp.tile([128, 2 * 256], bf16, tag="WczT")
            nc.sync.dma_start(out=WczT[:], in_=WczT_d[:])
            wfcm = vec(wfcm_d, 2, "wfcm", f32)
            bout = vec(bout_d, 2, "bout", f32)
            nc.sync.dma_start(out=xmt[1][:], in_=xm_d[1])
            WcsT = wp.tile([128, 2 * 256], bf16, tag="WcsT")
            nc.sync.dma_start(out=WcsT[:], in_=WcsT_d[:])
            wfcs = wp.tile([1, 256], bf16, tag="wfcs")
            nc.sync.dma_start(out=wfcs[:], in_=wfcs_d[:])
            fetch_xs(0, 1)

            # ================= main branches (32x32), b0/b1 stage-skewed =====
            def main_a(b):
                xm = xmt[b]
                # z = W_align @ x + (b_align + pos): the whole main branch
                # (score AND value) factors through this 256-dim space
                zt = mp.tile([128, 2 * NPX_M], bf16, tag="zt", bufs=2, name=f"zt{b}")
                for n in range(2):
                    for m in range(2):
                        py = ps_y.tile([128, 512], f32, tag="y")
                        for k in range(4):
                            nc.tensor.matmul(
                                py[:], WalT[:, k * 256 + m * 128:k * 256 + (m + 1) * 128],
                                xm[:, k * NPX_M + n * 512:k * NPX_M + (n + 1) * 512],
                                start=(k == 0), stop=(k == 3))
                        nc.scalar.activation(zt[:, m * NPX_M + n * 512:m * NPX_M + (n + 1) * 512],
                                             py[:], Act.Identity, bias=zbias[:, m:m + 1],
                                             scale=1.0)
                t2m = [mp.tile([128, NPX_M], f32r, tag=f"tm{m}", bufs=2,
                               name=f"tm{m}_{b}") for m in range(2)]
                smf = mp.tile([1, NPX_M], f32, tag="smf", bufs=2)
                for n in range(2):
                    for m in range(2):
                        py = ps_y.tile([128, 512], f32, tag="y")
                        for k in range(2):
                            nc.tensor.matmul(
                                py[:], WzT[:, k * 256 + m * 128:k * 256 + (m + 1) * 128],
                                zt[:, k * NPX_M + n * 512:k * NPX_M + (n + 1) * 512],
                                start=(k == 0), stop=(k == 1))
                        nc.scalar.activation(t2m[m][:, n * 512:(n + 1) * 512], py[:],
                                             Act.Square, bias=cm[:, m:m + 1], scale=1.0)
                    pscr = ps_s.tile([1, 512], f32, tag="s")
                    for k in range(2):
                        nc.tensor.matmul(pscr[:], sgm[:, k:k + 1],
                                         t2m[k][:, n * 512:(n + 1) * 512],
                                         start=(k == 0), stop=(k == 1))
                    nc.scalar.activation(smf[:, n * 512:(n + 1) * 512], pscr[:], Act.Copy)
                return smf, zt

            def main_bc(b, smf, zt):
                # softmax over w (32-wide rows, natural px order at 32x32)
                shw = mp.tile([32, 32], f32, tag="shw", bufs=2)
                nc.sync.dma_start(out=shw[:], in_=smf[:])
                nmax = mp.tile([32, 1], f32, tag="nmax", bufs=2)
                nc.vector.tensor_reduce(nmax[:], shw[:], axis=mybir.AxisListType.X,
                                        op=Alu.max, negate=True)
                ex = mp.tile([32, 32], f32, tag="ex", bufs=2)
                esum = mp.tile([32, 1], f32, tag="esum", bufs=2)
                nc.scalar.activation(ex[:], shw[:], Act.Exp, bias=nmax[:], scale=1.0,
                                     accum_out=esum[:])
                rec = mp.tile([32, 1], f32, tag="rec", bufs=2)
                nc.vector.reciprocal(rec[:], esum[:])
                attnm = mp.tile([32, 32], bf16, tag="attnm", bufs=2)
                nc.scalar.activation(attnm[:], ex[:], Act.Copy, scale=rec[:])
                amf = mp.tile([1, NPX_M], bf16, tag="amf", bufs=2)
                nc.sync.dma_start(out=amf[:], in_=attnm[:])
                amb = mp.tile([128, NPX_M], bf16, tag="amb", bufs=2)
                nc.gpsimd.partition_broadcast(amb[:], amf[:])

                # z'm = z * attn (bf16 2x TT); fused GEMM; fmb = amb*wfcm + pf
                xpm = mp.tile([128, 2 * NPX_M], bf16, tag="xpm", bufs=2)
                for n in range(2):
                    for k in range(2):
                        nc.vector.tensor_tensor(
                            xpm[:, k * NPX_M + n * 512:k * NPX_M + (n + 1) * 512],
                            zt[:, k * NPX_M + n * 512:k * NPX_M + (n + 1) * 512],
                            amb[:, n * 512:(n + 1) * 512], op=Alu.mult)
                for m in range(2):
                    fmb[(b, m)] = mp.tile([128, NPX_M], bf16, tag=f"fmb{b}{m}", bufs=1,
                                          name=f"fmb{b}{m}")
                for n in range(2):
                    for m in range(2):
                        pf = ps_f.tile([128, 512], f32, tag="f")
                        for k in range(2):
                            nc.tensor.matmul(
                                pf[:], WczT[:, k * 256 + m * 128:k * 256 + (m + 1) * 128],
                                xpm[:, k * NPX_M + n * 512:k * NPX_M + (n + 1) * 512],
                                start=(k == 0), stop=(k == 1))
                        nc.vector.scalar_tensor_tensor(
                            fmb[(b, m)][:, n * 512:(n + 1) * 512],
                            amb[:, n * 512:(n + 1) * 512], wfcm[:, m:m + 1], pf[:],
                            op0=Alu.mult, op1=Alu.add)

            smf0, zt0 = main_a(0)

            # ====== small branches; px' = (dh, h2l, dw, wj) within each chunk ======
            # Software-pipelined emission with a one-unit skew: stage A
            # (score GEMMs+Squares) of unit u is emitted before stage B
            # (softmax) and C (fused GEMMs) of unit u-1, so each in-order
            # engine queue always has ready work ahead of chain-latency ops.
            out_dmas = []
            pending_combines = []
            units = [(c, b) for c in range(NCHUNK) for b in range(BPC)]

            def stage_a(u):
                c, b = u
                if (c, b) not in xst_all:
                    fetch_xs(c, b)
                xst = xst_all[(c, b)]
                if c + 1 < NCHUNK and (c + 1, b) not in xst_all:
                    fetch_xs(c + 1, b)
                t2s = [sp.tile([128, CHUNK], f32r, tag=f"ts{m}", bufs=3,
                               name=f"ts{m}") for m in range(2)]
                sf = sp.tile([1, CHUNK], f32, tag="sf", bufs=3)
                for n in range(2):
                    for m in range(2):
                        py = ps_y.tile([128, 512], f32, tag="y")
                        for k in range(2):
                            nc.tensor.matmul(
                                py[:], WsT[:, k * 256 + m * 128:k * 256 + (m + 1) * 128],
                                xst[:, k * CHUNK + n * 512:k * CHUNK + (n + 1) * 512],
                                start=(k == 0), stop=(k == 1))
                        nc.scalar.activation(t2s[m][:, n * 512:(n + 1) * 512], py[:],
                                             Act.Square, bias=cs[:, m:m + 1], scale=1.0)
                    pscr = ps_s.tile([1, 512], f32, tag="s")
                    for k in range(2):
                        nc.tensor.matmul(pscr[:], sgs[:, k:k + 1],
                                         t2s[k][:, n * 512:(n + 1) * 512],
                                         start=(k == 0), stop=(k == 1))
                    nc.scalar.activation(sf[:, n * 512:(n + 1) * 512], pscr[:], Act.Copy)
                return {"xst": xst, "sf": sf}

            def stage_b(st):
                # softmax: rows (dh,h2l), cols (dw,wj). Each dh-half (8 rows
                # = one n-block) is independent, so the whole chain down to
                # the broadcast runs per-half and stage C's n0 GEMMs start a
                # broadcast-latency earlier.
                # halves live side-by-side on partitions 0-7 (engine ops
                # must start at partition 0)
                shw_s = sp.tile([8, 2 * 64], f32, tag="shw_s", bufs=4)
                nmax_s = sp.tile([8, 2], f32, tag="nmax_s", bufs=4)
                ex_s = sp.tile([8, 2 * 64], f32, tag="ex_s", bufs=4)
                esum_s = sp.tile([8, 2], f32, tag="esum_s", bufs=4)
                rec_s = sp.tile([8, 2], f32, tag="rec_s", bufs=4)
                attn_s = sp.tile([8, 2 * 64], bf16, tag="attn_s", bufs=4)
                af = sp.tile([1, CHUNK], bf16, tag="af", bufs=4)
                asb = sp.tile([128, CHUNK], bf16, tag="asb", bufs=2)
                for n in range(2):
                    ws = slice(n * 64, (n + 1) * 64)
                    cseg = slice(n * 512, (n + 1) * 512)
                    nc.sync.dma_start(out=shw_s[:, ws], in_=st["sf"][:, cseg])
                    nc.vector.tensor_reduce(nmax_s[:, n:n + 1], shw_s[:, ws],
                                            axis=mybir.AxisListType.X,
                                            op=Alu.max, negate=True)
                    nc.scalar.activation(ex_s[:, ws], shw_s[:, ws], Act.Exp,
                                         bias=nmax_s[:, n:n + 1], scale=1.0,
                                         accum_out=esum_s[:, n:n + 1])
                    nc.vector.reciprocal(rec_s[:, n:n + 1], esum_s[:, n:n + 1])
                    nc.scalar.activation(attn_s[:, ws], ex_s[:, ws], Act.Copy,
                                         scale=rec_s[:, n:n + 1])
                    nc.sync.dma_start(out=af[:, cseg], in_=attn_s[:, ws])
                    nc.gpsimd.partition_broadcast(asb[:, cseg], af[:, cseg])
                st["af"], st["asb"] = af, asb

            def stage_c(st, u):
                c, b = u
                xst, af, asb = st["xst"], st["af"], st["asb"]
                # x's = x * attn (bf16 2x TT)
                xps = sp.tile([128, 2 * CHUNK], bf16, tag="xps", bufs=3)
                for n in range(2):
                    for k in range(2):
                        nc.vector.tensor_tensor(
                            xps[:, k * CHUNK + n * 512:k * CHUNK + (n + 1) * 512],
                            xst[:, k * CHUNK + n * 512:k * CHUNK + (n + 1) * 512],
                            asb[:, n * 512:(n + 1) * 512], op=Alu.mult)
                # out = Wcs@x' + wfcs(x)attn (+bout +fmb_up in the combine)
                outc = sp.tile([128, 2 * CHUNK], bf16, tag="outc", bufs=3)
                pfs = {}
                for n in range(2):
                    for m in range(2):
                        pf = ps_f.tile([128, 512], f32, tag="f")
                        for k in range(2):
                            nc.tensor.matmul(
                                pf[:], WcsT[:, k * 256 + m * 128:k * 256 + (m + 1) * 128],
                                xps[:, k * CHUNK + n * 512:k * CHUNK + (n + 1) * 512],
                                start=(k == 0), stop=False)
                        nc.tensor.matmul(
                            pf[:], wfcs[0:1, m * 128:(m + 1) * 128],
                            af[0:1, n * 512:(n + 1) * 512],
                            start=False, stop=True)
                        pfs[(n, m)] = pf

                def emit_combine(outc=outc, pfs=pfs, c=c, b=b):
                    for m in range(2):
                        for n in range(2):
                            pf = pfs[(n, m)]
                            ov = outc[:, m * CHUNK + n * 512:m * CHUNK + (n + 1) * 512] \
                                .rearrange("p (h dw w) -> p h dw w", h=8, dw=2)
                            pv3 = pf[:].rearrange("p (h dw w) -> p h dw w", h=8, dw=2)
                            fv = fmb[(b, m)][:, c * 256:(c + 1) * 256] \
                                .rearrange("p (h w) -> p h w", h=8)
                            for dw in range(2):
                                nc.vector.scalar_tensor_tensor(
                                    ov[:, :, dw], pv3[:, :, dw], bout[:, m:m + 1],
                                    fv[:], op0=Alu.add, op1=Alu.add)
                    out_dmas.append((outc, c, b))

                pending_combines.append(emit_combine)
                if len(pending_combines) > 1:
                    pending_combines.pop(0)()

            # prologue: interleave the first small A-stages with the main
            # softmax/value stages so PE never waits on a softmax chain;
            # steady state runs a two-unit skew (A two ahead of B/C)
            queue = []

            def drain_one():
                uu, ss = queue.pop(0)
                stage_b(ss)
                stage_c(ss, uu)

            smf1, zt1 = main_a(1)
            queue.append((units[0], stage_a(units[0])))
            main_bc(0, smf0, zt0)
            queue.append((units[1], stage_a(units[1])))
            main_bc(1, smf1, zt1)
            for u in units[2:]:
                queue.append((u, stage_a(u)))
                if len(queue) > 2:
                    drain_one()
            while queue:
                drain_one()

            for fn in pending_combines:
                fn()

            for outc, c, b in out_dmas:
                for m in range(2):
                    nc.sync.dma_start(
                        out=out_d[b, :, m, c * CHUNK:(c + 1) * CHUNK],
                        in_=outc[:, m * CHUNK:(m + 1) * CHUNK])

    nc.compile()
    return nc


def _prep_weights(W_align, b_align, pos_embed_main, pos_embed_small,
                  W_qkv_s, b_qkv_s, W_proj_s, b_proj_s, rel_pos_s,
                  W_qkv_m, b_qkv_m, W_proj_m, b_proj_m, rel_pos_m,
                  W_fuse, b_fuse):
    d = np.float64
    W_align, b_align = W_align.astype(d), b_align.astype(d)
    pos_s = pos_embed_small.reshape(-1).astype(d)
    pos_m = pos_embed_main.reshape(-1).astype(d)
    rel_s = rel_pos_s.reshape(-1).astype(d)
    rel_m = rel_pos_m.reshape(-1).astype(d)
    W_qkv_s, b_qkv_s = W_qkv_s.astype(d), b_qkv_s.astype(d)
    W_qkv_m, b_qkv_m = W_qkv_m.astype(d), b_qkv_m.astype(d)
    W_proj_s, b_proj_s = W_proj_s.astype(d), b_proj_s.astype(d)
    W_proj_m, b_proj_m = W_proj_m.astype(d), b_proj_m.astype(d)
    W_fuse, b_fuse = W_fuse.astype(d), b_fuse.astype(d)
    scale = 1.0 / np.sqrt(np.float64(CS))

    def eig_form(A, u):
        n = A.shape[0]
        Bt = np.zeros((n + 1, n + 1))
        Bt[:n, :n] = (A + A.T) / 2
        Bt[:n, n] = u / 2
        Bt[n, :n] = u / 2
        lam, Q = np.linalg.eigh(Bt)
        keep = np.argsort(-np.abs(lam))[:n]
        lam, Q = lam[keep], Q[:, keep]
        Wt = np.sqrt(np.abs(lam))[:, None] * Q.T
        return Wt[:, :n], np.sign(lam), Wt[:, n]

    # small branch
    Wq, Wk, Wv = W_qkv_s[:CS], W_qkv_s[CS:2 * CS], W_qkv_s[2 * CS:]
    bq, bk, bv = b_qkv_s[:CS], b_qkv_s[CS:2 * CS], b_qkv_s[2 * CS:]
    cq = Wq @ pos_s + bq + rel_s
    ck = Wk @ pos_s + bk + rel_s
    cv_s = Wv @ pos_s + bv
    A_s = (Wq.T @ Wk) * scale
    u_s = (Wk.T @ cq + Wq.T @ ck) * scale
    Ws, sg_s, c_s = eig_form(A_s, u_s)

    # main branch in z-space: z = W_align x + b_align + pos (256-dim);
    # both score and value factor through z
    Wqm = W_qkv_m[:CS]
    Wkm = W_qkv_m[CS:2 * CS]
    Wvm = W_qkv_m[2 * CS:]
    cqm = b_qkv_m[:CS] + rel_m
    ckm = b_qkv_m[CS:2 * CS] + rel_m
    cv_m = b_qkv_m[2 * CS:]
    A_m = (Wqm.T @ Wkm) * scale
    u_m = (Wkm.T @ cqm + Wqm.T @ ckm) * scale
    Wm, sg_m, c_m = eig_form(A_m, u_m)
    zbias_v = b_align + pos_m

    # fuse folding (1/2 on main: upsampled softmax over 64 = low-res softmax/2)
    W_fs = W_fuse[:, :CS] @ W_proj_s
    W_fm = (W_fuse[:, CS:] @ W_proj_m) * 0.5
    b_out = b_fuse + W_fuse[:, :CS] @ b_proj_s + W_fuse[:, CS:] @ b_proj_m

    # fused attn-through-proj GEMMs + rank-1 constants
    W_cs = W_fs @ Wv            # [256, 256]
    wfc_s = W_fs @ cv_s         # [256]
    W_cm = W_fm @ Wvm           # [256, 256]  (z-space)
    wfc_m = W_fm @ cv_m         # [256]

    f = np.float32

    def packT(W, kblocks):
        rows, ch = W.shape
        assert ch == kblocks * 128
        t = W.T.reshape(kblocks, 128, rows).transpose(1, 0, 2).reshape(128, kblocks * rows)
        return np.ascontiguousarray(t.astype(BF16))

    def colvec(v, chunks):
        return np.ascontiguousarray(v.reshape(chunks, 128).T.astype(f))

    return {
        "WalT": packT(W_align, 4), "zbias": colvec(zbias_v, 2),
        "WzT": packT(Wm, 2), "sgm": colvec(sg_m, 2), "cm": colvec(c_m, 2),
        "WczT": packT(W_cm, 2), "wfcm": colvec(wfc_m, 2), "bout": colvec(b_out, 2),
        "WsT": packT(Ws, 2), "sgs": colvec(sg_s, 2), "cs": colvec(c_s, 2),
        "WcsT": packT(W_cs, 2),
        "wfcs": np.ascontiguousarray(wfc_s.reshape(1, 256).astype(BF16)),
    }


def kernel(**inputs):
    global _prog
    from concourse.bass_utils import run_bass_kernel_spmd

    small = np.asarray(inputs["small_feat"], dtype=np.float32).reshape(B, CS, NPX_S)
    main = np.asarray(inputs["main_feat"], dtype=np.float32).reshape(B, CM, NPX_M)
    w = _prep_weights(**{k: np.asarray(v) for k, v in inputs.items()
                         if k not in ("small_feat", "main_feat")})

    small_p = small[:, :, PERM].astype(BF16)                       # [B, 256, 4096]
    xs_pack = np.ascontiguousarray(
        small_p.reshape(B, 2, 128, NPX_S).transpose(0, 2, 1, 3))   # [B, 128, 2, 4096]
    xm_pack = np.ascontiguousarray(
        main.astype(BF16).reshape(B, 4, 128, NPX_M)
        .transpose(0, 2, 1, 3).reshape(B, 128, 4 * NPX_M))         # [B, 128, 4096]

    if _prog is None:
        _prog = _build_program()
    nc = _prog

    in_maps = []
    for c in range(NCORES):
        m = dict(w)
        m["xs"] = xs_pack[c * BPC:(c + 1) * BPC]
        m["xm"] = xm_pack[c * BPC:(c + 1) * BPC]
        in_maps.append(m)

    res = run_bass_kernel_spmd(nc, in_maps, list(range(NCORES)))
    out_p = np.concatenate([r["out"] for r in res.results], axis=0)  # [B,128,2,4096]
    out_p = out_p.astype(np.float32).transpose(0, 2, 1, 3).reshape(B, CS, NPX_S)
    out = np.empty((B, CS, NPX_S), np.float32)
    out[:, :, PERM] = out_p
    return out.reshape(B, CS, HS, WS)


# revision 51
# speedup vs baseline: 1.0537x; 1.0115x over previous
"""CrossScaleAttention Trainium2 kernel (v3).

Data-parallel over batch: 16 samples / 8 cores = 2 samples per core.

Algebraic restructuring (exact up to fp rounding):
  - 1x1 convs commute with nearest 2x upsample -> main branch runs at 32x32.
  - W_align folded into main qkv; pos/rel/biases folded into constants.
  - attention score via AUGMENTED EIGENDECOMPOSITION:
      score = x^T A x + u.x = [x;1]^T Btil [x;1],  Btil symmetric (C+1)^2.
      eigh(Btil), keep C largest-|lam| comps (drops ~1e-6 residual):
      score = sum_i sgn_i * (W_i.x + c_i)^2 + const(dropped by softmax).
    Square runs on the Activation engine with c as per-partition bias.
  - attn scales pixels (columns), so it commutes through the proj GEMM:
      Wf @ ((Wv x + cv) * attn) = (Wf Wv) @ (x * attn) + (Wf cv) (x) attn
    One fused GEMM (Wc = Wf Wv) on x' = x*attn (a 2x-rate bf16 DVE multiply)
    plus a rank-1 matmul (lhsT = Wf cv, rhs = attn row) into the same PSUM.
  - bout folded into the small-branch combine stt; main fmb = (amb*wfcm)+pf.
  - small-branch pixels host-permuted to (dh, h2l, dw, wj) so the softmax
    transpose DMAs are flat copies and the 2x2 upsample-add segments are
    contiguous-innermost views of PSUM / fmb.
  - bf16 inputs/weights/attn/outputs; f32r square/score path (t' needs f32
    mantissa); matmuls accumulate in f32 PSUM.
  - out-DMAs emitted after all input DMAs on the in-order SP queue so their
    waits never block input prefetch.
"""
import sys
sys.path.insert(0, '/opt/trn_rl_repo')
import numpy as np
import ml_dtypes

B, CS, CM = 16, 256, 512
HS = WS = 64
HM = WM = 32
NPX_S = HS * WS          # 4096
NPX_M = HM * WM          # 1024
NCORES = 8
BPC = B // NCORES        # 2 samples per core
CHUNK = 1024             # small-branch pixel chunk (16 h-rows)
NCHUNK = NPX_S // CHUNK  # 4

BF16 = ml_dtypes.bfloat16


# pixel permutation within a chunk: px' = (dh, h2l, dw, wj), px = h*64+w with
# h = c*16 + 2*h2l + dh, w = 2*wj + dw  (c = chunk index)
def _chunk_perm():
    dh, h2l, dw, wj = np.meshgrid(np.arange(2), np.arange(8), np.arange(2),
                                  np.arange(32), indexing='ij')
    h = 2 * h2l + dh
    w = 2 * wj + dw
    return (h * 64 + w).reshape(-1)


_PERM_CHUNK = _chunk_perm()
PERM = np.concatenate([c * CHUNK + _PERM_CHUNK for c in range(NCHUNK)])  # [4096]

_prog = None


def _build_program():
    import concourse.bacc as bacc
    import concourse.mybir as mybir
    from concourse.tile import TileContext

    f32, f32r, bf16 = mybir.dt.float32, mybir.dt.float32r, mybir.dt.bfloat16
    Act = mybir.ActivationFunctionType
    Alu = mybir.AluOpType

    nc = bacc.Bacc(None, target_bir_lowering=False)

    xs_d = nc.dram_tensor("xs", [BPC, 128, 2, NPX_S], bf16, kind="ExternalInput")
    xm_d = nc.dram_tensor("xm", [BPC, 128, 4 * NPX_M], bf16, kind="ExternalInput")
    WalT_d = nc.dram_tensor("WalT", [128, 4 * 256], bf16, kind="ExternalInput")
    WzT_d = nc.dram_tensor("WzT", [128, 2 * 256], bf16, kind="ExternalInput")
    WczT_d = nc.dram_tensor("WczT", [128, 2 * 256], bf16, kind="ExternalInput")
    WsT_d = nc.dram_tensor("WsT", [128, 2 * 256], bf16, kind="ExternalInput")
    WcsT_d = nc.dram_tensor("WcsT", [128, 2 * 256], bf16, kind="ExternalInput")
    sgm_d = nc.dram_tensor("sgm", [128, 2], f32r, kind="ExternalInput")
    cm_d = nc.dram_tensor("cm", [128, 2], f32, kind="ExternalInput")
    wfcm_d = nc.dram_tensor("wfcm", [128, 2], f32, kind="ExternalInput")
    bout_d = nc.dram_tensor("bout", [128, 2], f32, kind="ExternalInput")
    sgs_d = nc.dram_tensor("sgs", [128, 2], f32r, kind="ExternalInput")
    cs_d = nc.dram_tensor("cs", [128, 2], f32, kind="ExternalInput")
    wfcs_d = nc.dram_tensor("wfcs", [1, 256], bf16, kind="ExternalInput")
    out_d = nc.dram_tensor("out", [BPC, 128, 2, NPX_S], bf16, kind="ExternalOutput")

    with TileContext(nc) as tc:
        with (
            tc.tile_pool(name="wp", bufs=1) as wp,
            tc.tile_pool(name="mp", bufs=1) as mp,
            tc.tile_pool(name="sp", bufs=1) as sp,
            tc.tile_pool(name="ps_y", bufs=3, space="PSUM") as ps_y,
            tc.tile_pool(name="ps_f", bufs=3, space="PSUM") as ps_f,
            tc.tile_pool(name="ps_s", bufs=2, space="PSUM") as ps_s,
        ):
            # ---- resident weights; k-interleaved so the first matmul can
            # start after ~190KB instead of the full main working set
            WalT = wp.tile([128, 4 * 256], bf16, tag="WalT")
            xmt = []
            for b in range(BPC):
                t = mp.tile([128, 4 * NPX_M], bf16, tag="xm", bufs=2, name=f"xm{b}")
                xmt.append(t)
            for k in range(4):
                nc.sync.dma_start(out=WalT[:, k * 256:(k + 1) * 256],
                                  in_=WalT_d[:, k * 256:(k + 1) * 256])
                nc.sync.dma_start(out=xmt[0][:, k * NPX_M:(k + 1) * NPX_M],
                                  in_=xm_d[0, :, k * NPX_M:(k + 1) * NPX_M])

            def vec(dram, cols, name, dt):
                t = wp.tile([128, cols], dt, tag=name)
                nc.sync.dma_start(out=t[:], in_=dram[:])
                return t

            WzT = wp.tile([128, 2 * 256], bf16, tag="WzT")
            nc.sync.dma_start(out=WzT[:], in_=WzT_d[:])
            sgm = vec(sgm_d, 2, "sgm", f32r)
            cm = vec(cm_d, 2, "cm", f32)
            # small-branch score weights early: lets the scheduler fill main-
            # phase PE bubbles with chunk-0 GEMMs
            WsT = wp.tile([128, 2 * 256], bf16, tag="WsT")
            nc.sync.dma_start(out=WsT[:], in_=WsT_d[:])
            sgs = vec(sgs_d, 2, "sgs", f32r)
            cs = vec(cs_d, 2, "cs", f32)

            fmb = {}  # (b, m) -> [128, 1024] bf16, persists into small phase

            xst_all = {}

            def fetch_xs(c, b):
                t = sp.tile([128, 2 * CHUNK], bf16, tag="xs", bufs=8,
                            name=f"xs{c}{b}")
                nc.sync.dma_start(
                    out=t[:].rearrange("p (k x) -> p k x", k=2),
                    in_=xs_d[b, :, :, c * CHUNK:(c + 1) * CHUNK])
                xst_all[(c, b)] = t

            fetch_xs(0, 0)
            WczT = wp.tile([128, 2 * 256], bf16, tag="WczT")
            nc.sync.dma_start(out=WczT[:], in_=WczT_d[:])
            wfcm = vec(wfcm_d, 2, "wfcm", f32)
            bout = vec(bout_d, 2, "bout", f32)
            nc.sync.dma_start(out=xmt[1][:], in_=xm_d[1])
            WcsT = wp.tile([128, 2 * 256], bf16, tag="WcsT")
            nc.sync.dma_start(out=WcsT[:], in_=WcsT_d[:])
            wfcs = wp.tile([1, 256], bf16, tag="wfcs")
            nc.sync.dma_start(out=wfcs[:], in_=wfcs_d[:])
            fetch_xs(0, 1)

            # ================= main branches (32x32), b0/b1 stage-skewed =====
            def main_a(b):
                xm = xmt[b]
                # z = W_align @ x + (b_align + pos): the whole main branch
                # (score AND value) factors through this 256-dim space
                zt = mp.tile([128, 2 * NPX_M], bf16, tag="zt", bufs=2, name=f"zt{b}")
                for n in range(2):
                    for m in range(2):
                        py = ps_y.tile([128, 512], f32, tag="y")
                        for k in range(4):
                            nc.tensor.matmul(
                                py[:], WalT[:, k * 256 + m * 128:k * 256 + (m + 1) * 128],
                                xm[:, k * NPX_M + n * 512:k * NPX_M + (n + 1) * 512],
                                start=(k == 0), stop=(k == 3))
                        nc.vector.tensor_copy(
                            zt[:, m * NPX_M + n * 512:m * NPX_M + (n + 1) * 512], py[:])
                t2m = [mp.tile([128, NPX_M], f32r, tag=f"tm{m}", bufs=2,
                               name=f"tm{m}_{b}") for m in range(2)]
                smf = mp.tile([1, NPX_M], f32, tag="smf", bufs=2)
                for n in range(2):
                    for m in range(2):
                        py = ps_y.tile([128, 512], f32, tag="y")
                        for k in range(2):
                            nc.tensor.matmul(
                                py[:], WzT[:, k * 256 + m * 128:k * 256 + (m + 1) * 128],
                                zt[:, k * NPX_M + n * 512:k * NPX_M + (n + 1) * 512],
                                start=(k == 0), stop=(k == 1))
                        nc.scalar.activation(t2m[m][:, n * 512:(n + 1) * 512], py[:],
                                             Act.Square, bias=cm[:, m:m + 1], scale=1.0)
                    pscr = ps_s.tile([1, 512], f32, tag="s")
                    for k in range(2):
                        nc.tensor.matmul(pscr[:], sgm[:, k:k + 1],
                                         t2m[k][:, n * 512:(n + 1) * 512],
                                         start=(k == 0), stop=(k == 1))
                    nc.scalar.activation(smf[:, n * 512:(n + 1) * 512], pscr[:], Act.Copy)
                return smf, zt

            def main_bc(b, smf, zt):
                # softmax over w (32-wide rows, natural px order at 32x32)
                shw = mp.tile([32, 32], f32, tag="shw", bufs=2)
                nc.sync.dma_start(out=shw[:], in_=smf[:])
                nmax = mp.tile([32, 1], f32, tag="nmax", bufs=2)
                nc.vector.tensor_reduce(nmax[:], shw[:], axis=mybir.AxisListType.X,
                                        op=Alu.max, negate=True)
                ex = mp.tile([32, 32], f32, tag="ex", bufs=2)
                esum = mp.tile([32, 1], f32, tag="esum", bufs=2)
                nc.scalar.activation(ex[:], shw[:], Act.Exp, bias=nmax[:], scale=1.0,
                                     accum_out=esum[:])
                rec = mp.tile([32, 1], f32, tag="rec", bufs=2)
                nc.vector.reciprocal(rec[:], esum[:])
                attnm = mp.tile([32, 32], bf16, tag="attnm", bufs=2)
                nc.scalar.activation(attnm[:], ex[:], Act.Copy, scale=rec[:])
                amf = mp.tile([1, NPX_M], bf16, tag="amf", bufs=2)
                nc.sync.dma_start(out=amf[:], in_=attnm[:])
                amb = mp.tile([128, NPX_M], bf16, tag="amb", bufs=2)
                nc.gpsimd.partition_broadcast(amb[:], amf[:])

                # z'm = z * attn (bf16 2x TT); fused GEMM; fmb = amb*wfcm + pf
                xpm = mp.tile([128, 2 * NPX_M], bf16, tag="xpm", bufs=2)
                for n in range(2):
                    for k in range(2):
                        nc.vector.tensor_tensor(
                            xpm[:, k * NPX_M + n * 512:k * NPX_M + (n + 1) * 512],
                            zt[:, k * NPX_M + n * 512:k * NPX_M + (n + 1) * 512],
                            amb[:, n * 512:(n + 1) * 512], op=Alu.mult)
                for m in range(2):
                    fmb[(b, m)] = mp.tile([128, NPX_M], bf16, tag=f"fmb{b}{m}", bufs=1,
                                          name=f"fmb{b}{m}")
                for n in range(2):
                    for m in range(2):
                        pf = ps_f.tile([128, 512], f32, tag="f")
                        for k in range(2):
                            nc.tensor.matmul(
                                pf[:], WczT[:, k * 256 + m * 128:k * 256 + (m + 1) * 128],
                                xpm[:, k * NPX_M + n * 512:k * NPX_M + (n + 1) * 512],
                                start=(k == 0), stop=(k == 1))
                        nc.vector.scalar_tensor_tensor(
                            fmb[(b, m)][:, n * 512:(n + 1) * 512],
                            amb[:, n * 512:(n + 1) * 512], wfcm[:, m:m + 1], pf[:],
                            op0=Alu.mult, op1=Alu.add)

            smf0, zt0 = main_a(0)

            # ====== small branches; px' = (dh, h2l, dw, wj) within each chunk ======
            # Software-pipelined emission with a one-unit skew: stage A
            # (score GEMMs+Squares) of unit u is emitted before stage B
            # (softmax) and C (fused GEMMs) of unit u-1, so each in-order
            # engine queue always has ready work ahead of chain-latency ops.
            out_dmas = []
            pending_combines = []
            units = [(c, b) for c in range(NCHUNK) for b in range(BPC)]

            def stage_a(u):
                c, b = u
                if (c, b) not in xst_all:
                    fetch_xs(c, b)
                xst = xst_all[(c, b)]
                if c + 1 < NCHUNK and (c + 1, b) not in xst_all:
                    fetch_xs(c + 1, b)
                t2s = [sp.tile([128, CHUNK], f32r, tag=f"ts{m}", bufs=3,
                               name=f"ts{m}") for m in range(2)]
                sf = sp.tile([1, CHUNK], f32, tag="sf", bufs=3)
                for n in range(2):
                    for m in range(2):
                        py = ps_y.tile([128, 512], f32, tag="y")
                        for k in range(2):
                            nc.tensor.matmul(
                                py[:], WsT[:, k * 256 + m * 128:k * 256 + (m + 1) * 128],
                                xst[:, k * CHUNK + n * 512:k * CHUNK + (n + 1) * 512],
                                start=(k == 0), stop=(k == 1))
                        nc.scalar.activation(t2s[m][:, n * 512:(n + 1) * 512], py[:],
                                             Act.Square, bias=cs[:, m:m + 1], scale=1.0)
                    pscr = ps_s.tile([1, 512], f32, tag="s")
                    for k in range(2):
                        nc.tensor.matmul(pscr[:], sgs[:, k:k + 1],
                                         t2s[k][:, n * 512:(n + 1) * 512],
                                         start=(k == 0), stop=(k == 1))
                    nc.scalar.activation(sf[:, n * 512:(n + 1) * 512], pscr[:], Act.Copy)
                return {"xst": xst, "sf": sf}

            def stage_b(st):
                # softmax: rows (dh,h2l), cols (dw,wj). Each dh-half (8 rows
                # = one n-block) is independent, so the whole chain down to
                # the broadcast runs per-half and stage C's n0 GEMMs start a
                # broadcast-latency earlier.
                # halves live side-by-side on partitions 0-7 (engine ops
                # must start at partition 0)
                shw_s = sp.tile([8, 2 * 64], f32, tag="shw_s", bufs=4)
                nmax_s = sp.tile([8, 2], f32, tag="nmax_s", bufs=4)
                ex_s = sp.tile([8, 2 * 64], f32, tag="ex_s", bufs=4)
                esum_s = sp.tile([8, 2], f32, tag="esum_s", bufs=4)
                rec_s = sp.tile([8, 2], f32, tag="rec_s", bufs=4)
                attn_s = sp.tile([8, 2 * 64], bf16, tag="attn_s", bufs=4)
                af = sp.tile([1, CHUNK], bf16, tag="af", bufs=4)
                asb = sp.tile([128, CHUNK], bf16, tag="asb", bufs=2)
                for n in range(2):
                    ws = slice(n * 64, (n + 1) * 64)
                    cseg = slice(n * 512, (n + 1) * 512)
                    nc.sync.dma_start(out=shw_s[:, ws], in_=st["sf"][:, cseg])
                    nc.vector.tensor_reduce(nmax_s[:, n:n + 1], shw_s[:, ws],
                                            axis=mybir.AxisListType.X,
                                            op=Alu.max, negate=True)
                    nc.scalar.activation(ex_s[:, ws], shw_s[:, ws], Act.Exp,
                                         bias=nmax_s[:, n:n + 1], scale=1.0,
                                         accum_out=esum_s[:, n:n + 1])
                    nc.vector.reciprocal(rec_s[:, n:n + 1], esum_s[:, n:n + 1])
                    nc.scalar.activation(attn_s[:, ws], ex_s[:, ws], Act.Copy,
                                         scale=rec_s[:, n:n + 1])
                    nc.sync.dma_start(out=af[:, cseg], in_=attn_s[:, ws])
                    nc.gpsimd.partition_broadcast(asb[:, cseg], af[:, cseg])
                st["af"], st["asb"] = af, asb

            def stage_c(st, u):
                c, b = u
                xst, af, asb = st["xst"], st["af"], st["asb"]
                # x's = x * attn (bf16 2x TT)
                xps = sp.tile([128, 2 * CHUNK], bf16, tag="xps", bufs=3)
                for n in range(2):
                    for k in range(2):
                        nc.vector.tensor_tensor(
                            xps[:, k * CHUNK + n * 512:k * CHUNK + (n + 1) * 512],
                            xst[:, k * CHUNK + n * 512:k * CHUNK + (n + 1) * 512],
                            asb[:, n * 512:(n + 1) * 512], op=Alu.mult)
                # out = Wcs@x' + wfcs(x)attn (+bout +fmb_up in the combine)
                outc = sp.tile([128, 2 * CHUNK], bf16, tag="outc", bufs=3)
                pfs = {}
                for n in range(2):
                    for m in range(2):
                        pf = ps_f.tile([128, 512], f32, tag="f")
                        for k in range(2):
                            nc.tensor.matmul(
                                pf[:], WcsT[:, k * 256 + m * 128:k * 256 + (m + 1) * 128],
                                xps[:, k * CHUNK + n * 512:k * CHUNK + (n + 1) * 512],
                                start=(k == 0), stop=False)
                        nc.tensor.matmul(
                            pf[:], wfcs[0:1, m * 128:(m + 1) * 128],
                            af[0:1, n * 512:(n + 1) * 512],
                            start=False, stop=True)
                        pfs[(n, m)] = pf

                def emit_combine(outc=outc, pfs=pfs, c=c, b=b):
                    for m in range(2):
                        for n in range(2):
                            pf = pfs[(n, m)]
                            ov = outc[:, m * CHUNK + n * 512:m * CHUNK + (n + 1) * 512] \
                                .rearrange("p (h dw w) -> p h dw w", h=8, dw=2)
                            pv3 = pf[:].rearrange("p (h dw w) -> p h dw w", h=8, dw=2)
                            fv = fmb[(b, m)][:, c * 256:(c + 1) * 256] \
                                .rearrange("p (h w) -> p h w", h=8)
                            for dw in range(2):
                                nc.vector.scalar_tensor_tensor(
                                    ov[:, :, dw], pv3[:, :, dw], bout[:, m:m + 1],
                                    fv[:], op0=Alu.add, op1=Alu.add)
                    out_dmas.append((outc, c, b))

                pending_combines.append(emit_combine)
                if len(pending_combines) > 1:
                    pending_combines.pop(0)()

            # prologue: interleave the first small A-stages with the main
            # softmax/value stages so PE never waits on a softmax chain;
            # steady state runs a two-unit skew (A two ahead of B/C)
            queue = []

            def drain_one():
                uu, ss = queue.pop(0)
                stage_b(ss)
                stage_c(ss, uu)

            smf1, zt1 = main_a(1)
            queue.append((units[0], stage_a(units[0])))
            main_bc(0, smf0, zt0)
            queue.append((units[1], stage_a(units[1])))
            main_bc(1, smf1, zt1)
            for u in units[2:]:
                queue.append((u, stage_a(u)))
                if len(queue) > 2:
                    drain_one()
            while queue:
                drain_one()

            for fn in pending_combines:
                fn()

            for outc, c, b in out_dmas:
                for m in range(2):
                    nc.sync.dma_start(
                        out=out_d[b, :, m, c * CHUNK:(c + 1) * CHUNK],
                        in_=outc[:, m * CHUNK:(m + 1) * CHUNK])

    nc.compile()
    return nc


def _prep_weights(W_align, b_align, pos_embed_main, pos_embed_small,
                  W_qkv_s, b_qkv_s, W_proj_s, b_proj_s, rel_pos_s,
                  W_qkv_m, b_qkv_m, W_proj_m, b_proj_m, rel_pos_m,
                  W_fuse, b_fuse):
    d = np.float64
    W_align, b_align = W_align.astype(d), b_align.astype(d)
    pos_s = pos_embed_small.reshape(-1).astype(d)
    pos_m = pos_embed_main.reshape(-1).astype(d)
    rel_s = rel_pos_s.reshape(-1).astype(d)
    rel_m = rel_pos_m.reshape(-1).astype(d)
    W_qkv_s, b_qkv_s = W_qkv_s.astype(d), b_qkv_s.astype(d)
    W_qkv_m, b_qkv_m = W_qkv_m.astype(d), b_qkv_m.astype(d)
    W_proj_s, b_proj_s = W_proj_s.astype(d), b_proj_s.astype(d)
    W_proj_m, b_proj_m = W_proj_m.astype(d), b_proj_m.astype(d)
    W_fuse, b_fuse = W_fuse.astype(d), b_fuse.astype(d)
    scale = 1.0 / np.sqrt(np.float64(CS))

    def eig_form(A, u):
        n = A.shape[0]
        Bt = np.zeros((n + 1, n + 1))
        Bt[:n, :n] = (A + A.T) / 2
        Bt[:n, n] = u / 2
        Bt[n, :n] = u / 2
        lam, Q = np.linalg.eigh(Bt)
        keep = np.argsort(-np.abs(lam))[:n]
        lam, Q = lam[keep], Q[:, keep]
        Wt = np.sqrt(np.abs(lam))[:, None] * Q.T
        return Wt[:, :n], np.sign(lam), Wt[:, n]

    # small branch
    Wq, Wk, Wv = W_qkv_s[:CS], W_qkv_s[CS:2 * CS], W_qkv_s[2 * CS:]
    bq, bk, bv = b_qkv_s[:CS], b_qkv_s[CS:2 * CS], b_qkv_s[2 * CS:]
    cq = Wq @ pos_s + bq + rel_s
    ck = Wk @ pos_s + bk + rel_s
    cv_s = Wv @ pos_s + bv
    A_s = (Wq.T @ Wk) * scale
    u_s = (Wk.T @ cq + Wq.T @ ck) * scale
    Ws, sg_s, c_s = eig_form(A_s, u_s)

    # main branch in z-space: z = W_align x + b_align + pos (256-dim);
    # both score and value factor through z
    Wqm = W_qkv_m[:CS]
    Wkm = W_qkv_m[CS:2 * CS]
    Wvm = W_qkv_m[2 * CS:]
    cqm = b_qkv_m[:CS] + rel_m
    ckm = b_qkv_m[CS:2 * CS] + rel_m
    cv_m = b_qkv_m[2 * CS:]
    A_m = (Wqm.T @ Wkm) * scale
    u_m = (Wkm.T @ cqm + Wqm.T @ ckm) * scale
    Wm, sg_m, c_m = eig_form(A_m, u_m)
    # z is computed WITHOUT its bias on-chip; fold zb into the score bias
    # (c += Wm @ zb) and into the value rank-1 constant (wfc += Wcm @ zb)
    zb = b_align + pos_m
    c_m = c_m + Wm @ zb

    # fuse folding (1/2 on main: upsampled softmax over 64 = low-res softmax/2)
    W_fs = W_fuse[:, :CS] @ W_proj_s
    W_fm = (W_fuse[:, CS:] @ W_proj_m) * 0.5
    b_out = b_fuse + W_fuse[:, :CS] @ b_proj_s + W_fuse[:, CS:] @ b_proj_m

    # fused attn-through-proj GEMMs + rank-1 constants
    W_cs = W_fs @ Wv            # [256, 256]
    wfc_s = W_fs @ cv_s         # [256]
    W_cm = W_fm @ Wvm           # [256, 256]  (z-space)
    wfc_m = W_fm @ cv_m + W_cm @ zb   # [256]

    f = np.float32

    def packT(W, kblocks):
        rows, ch = W.shape
        assert ch == kblocks * 128
        t = W.T.reshape(kblocks, 128, rows).transpose(1, 0, 2).reshape(128, kblocks * rows)
        return np.ascontiguousarray(t.astype(BF16))

    def colvec(v, chunks):
        return np.ascontiguousarray(v.reshape(chunks, 128).T.astype(f))

    return {
        "WalT": packT(W_align, 4),
        "WzT": packT(Wm, 2), "sgm": colvec(sg_m, 2), "cm": colvec(c_m, 2),
        "WczT": packT(W_cm, 2), "wfcm": colvec(wfc_m, 2), "bout": colvec(b_out, 2),
        "WsT": packT(Ws, 2), "sgs": colvec(sg_s, 2), "cs": colvec(c_s, 2),
        "WcsT": packT(W_cs, 2),
        "wfcs": np.ascontiguousarray(wfc_s.reshape(1, 256).astype(BF16)),
    }


def kernel(**inputs):
    global _prog
    from concourse.bass_utils import run_bass_kernel_spmd

    small = np.asarray(inputs["small_feat"], dtype=np.float32).reshape(B, CS, NPX_S)
    main = np.asarray(inputs["main_feat"], dtype=np.float32).reshape(B, CM, NPX_M)
    w = _prep_weights(**{k: np.asarray(v) for k, v in inputs.items()
                         if k not in ("small_feat", "main_feat")})

    small_p = small[:, :, PERM].astype(BF16)                       # [B, 256, 4096]
    xs_pack = np.ascontiguousarray(
        small_p.reshape(B, 2, 128, NPX_S).transpose(0, 2, 1, 3))   # [B, 128, 2, 4096]
    xm_pack = np.ascontiguousarray(
        main.astype(BF16).reshape(B, 4, 128, NPX_M)
        .transpose(0, 2, 1, 3).reshape(B, 128, 4 * NPX_M))         # [B, 128, 4096]

    if _prog is None:
        _prog = _build_program()
    nc = _prog

    in_maps = []
    for c in range(NCORES):
        m = dict(w)
        m["xs"] = xs_pack[c * BPC:(c + 1) * BPC]
        m["xm"] = xm_pack[c * BPC:(c + 1) * BPC]
        in_maps.append(m)

    res = run_bass_kernel_spmd(nc, in_maps, list(range(NCORES)))
    out_p = np.concatenate([r["out"] for r in res.results], axis=0)  # [B,128,2,4096]
    out_p = out_p.astype(np.float32).transpose(0, 2, 1, 3).reshape(B, CS, NPX_S)
    out = np.empty((B, CS, NPX_S), np.float32)
    out[:, :, PERM] = out_p
    return out.reshape(B, CS, HS, WS)
